# revision 1
# baseline (speedup 1.0000x reference)
"""HGNN layer kernel for 8 Trainium2 NeuronCores — batched SWDGE version.

out = x @ C_w.T + C_b + sum_t scatter_add(dest_t, (1/counts_t[dest]) * msg_t)
msg_t[g] = concat(x[src_{g,k}] for k in arity) @ A_t

Key mechanisms (all verified by microbenchmarks on this stack):
  - dma_gather (InstDMAGatherAnt): batched indexed row gather, <= carveout
    descriptors per call (dynamic_dma_scratch_size/16), int16 indices
    relative to a sliced in_ap base (windows of 25000 rows), indices packed
    [i%16, i//16] and replicated across the 8 gpsimd Q7 cores.
  - dma_scatter_add (InstDMAScatterAddAnt): batched CCE scatter-add. NOT
    atomic for duplicate dests (neither within nor across calls) ->
    occurrence rounds: each call holds unique dests; rounds are sem-gated.
  - Per-entry decomposition: msg_group = sum_k x[src_k] @ A_block, so each
    (group, slot) entry is gathered / matmul'd / scattered independently;
    CCE adds do the k-summation and the cross-type aggregation in DRAM.
  - bf16 x / A (PSUM accumulate f32), f32 scatter payload.
  - Residual (x @ C_w.T + C_b) computed per-shard with plain DMA loads and
    written to out rows first; scatters are gated on its completion.

Entry stream layout (per core, uniform shape across cores):
  entries sorted by (round, window(src), block); each (round, window) span
  padded to a multiple of 128 (pad: src=window base, dest=trash row,
  scale=0).  Chunks of 128 entries map to plane position [i%128, i//128].
"""

import sys

for _p in ("/opt/trn_rl_repo",):
    if _p not in sys.path:
        sys.path.insert(0, _p)

import numpy as np
import ml_dtypes

import concourse.bass as bass
import concourse.bacc as bacc
import concourse.mybir as mybir
import concourse.tile as tile
from concourse.bass_utils import run_bass_kernel_spmd
from concourse.library_config import mlp
from concourse.masks import make_identity
from concourse.tile import add_dep_helper

N_CORES = 8
D = 128
P = 128
N_NODES = 100000
SHARD = N_NODES // N_CORES  # 12500
WIN = 25000                 # gather window rows (int16 index limit 32767)
NWIN = 4
SCRATCH = 16384             # dynamic dma scratch (carveout = SCRATCH/16 descs)
GCALL = 1024                # max descriptors per dma_gather call
SCALL = 1024                # max descriptors per dma_scatter_add call
STRIP = 4                   # chunks per psum strip (512 entries)
ACC_STRIDE = SHARD + P      # one accumulator region (incl trash rows)
N_ACC = 2                   # accumulators (halves occurrence rounds)
OUT_ROWS = N_ACC * ACC_STRIDE

TRACE = False
LAST_RESULTS = None

BLOCKS = 6  # (t0,k0) (t1,k0) (t1,k1) (t2,k0) (t2,k1) (t2,k2)


# --------------------------------------------------------------------------
# Host-side preparation
# --------------------------------------------------------------------------

def _idx_plane16(vals, length):
    """int16 plane [128, length//16]: entry i -> [i%16, i//16], replicated
    across the 8 gpsimd cores."""
    assert length % 16 == 0
    v = np.zeros(length, np.int16)
    v[: len(vals)] = vals
    pl = v.reshape(length // 16, 16).T  # [16, C]
    return np.ascontiguousarray(np.tile(pl, (8, 1)))


def _plane128(vals, length, dtype):
    """plane [128, length//128]: entry i -> [i%128, i//128]."""
    assert length % 128 == 0
    v = np.zeros(length, dtype)
    v[: len(vals)] = vals
    return np.ascontiguousarray(v.reshape(length // 128, 128).T)


def _prep(edges):
    """Build the uniform entry-stream layout for all cores.

    Returns (layout, per_core) where layout holds the uniform geometry
    (span lengths per (round, window), gather-call and scatter-call tables,
    chunk block-segment table) and per_core holds index/scale planes.
    """
    # global per-type degree -> scale
    entries_core = [[] for _ in range(N_CORES)]  # (block, src, dloc, scale)
    for t, (e, arity) in enumerate(edges):
        e = np.asarray(e)
        g = e.shape[1] // arity
        dest = e[1].reshape(g, arity)[:, 0].astype(np.int64)
        srcs = e[0].reshape(g, arity).astype(np.int64)
        counts = np.bincount(dest, minlength=N_NODES)
        inv = np.zeros(N_NODES, np.float32)
        nz = counts > 0
        inv[nz] = np.float32(1.0) / counts[nz].astype(np.float32)
        blk0 = {0: 0, 1: 1, 2: 3}[t]
        core_of = dest // SHARD
        for c in range(N_CORES):
            sel = np.where(core_of == c)[0]
            d_loc = dest[sel] - c * SHARD
            sc = inv[dest[sel]]
            for k in range(arity):
                entries_core[c].append(
                    (blk0 + k, srcs[sel, k], d_loc, sc)
                )

    # per core: arrays (block, src, dloc, scale), then round assignment
    core_arrays = []
    max_rounds = 0
    for c in range(N_CORES):
        blocks = np.concatenate(
            [np.full(len(s), b, np.int32) for b, s, d, v in entries_core[c]])
        srcs = np.concatenate([s for b, s, d, v in entries_core[c]])
        dls = np.concatenate([d for b, s, d, v in entries_core[c]])
        scs = np.concatenate([v for b, s, d, v in entries_core[c]])
        # occurrence round per dest (order of entries within dest arbitrary)
        order = np.argsort(dls, kind="stable")
        d_sorted = dls[order]
        first = np.searchsorted(d_sorted, d_sorted, side="left")
        occ = np.arange(len(d_sorted)) - first
        occ_all = np.empty(len(dls), np.int64)
        occ_all[order] = occ
        accs = (occ_all % N_ACC).astype(np.int64)
        rounds = (occ_all // N_ACC).astype(np.int32)
        dls = dls + accs * ACC_STRIDE  # absolute dest row incl accumulator
        max_rounds = max(max_rounds, int(rounds.max()) + 1 if len(rounds) else 0)
        core_arrays.append((blocks, srcs, dls, scs, rounds))

    R = max_rounds
    # uniform span lengths per (round, window): max over cores, pad to 128
    span_len = np.zeros((R, NWIN), np.int64)
    core_spans = []  # per core: dict (r,w) -> index arrays sorted by block
    for c in range(N_CORES):
        blocks, srcs, dls, scs, rounds = core_arrays[c]
        w = srcs // WIN
        # sort by (round, window, block) stable
        key = ((rounds.astype(np.int64) * NWIN + w) * BLOCKS
               + blocks.astype(np.int64))
        order = np.argsort(key, kind="stable")
        spans = {}
        ks = key[order]
        # boundaries of each (r,w) pair
        rw = ks // BLOCKS
        uniq, starts = np.unique(rw, return_index=True)
        starts = list(starts) + [len(ks)]
        for i, u in enumerate(uniq):
            r, wi = divmod(int(u), NWIN)
            idx = order[starts[i]: starts[i + 1]]
            spans[(r, wi)] = idx
            span_len[r, wi] = max(span_len[r, wi], len(idx))
        core_spans.append(spans)
    span_len = ((span_len + 127) // 128) * 128

    # layout tables (uniform): stream positions of spans, gather calls,
    # scatter calls, chunk block segments (block id per 128-chunk segment
    # must be uniform? blocks differ per core within a chunk!).
    #
    # NOTE: the A-matmul per chunk needs per-core-uniform block segments.
    # Within a span, entries are sorted by block, but per-core block counts
    # differ. To keep the device program uniform we ALSO make the per-
    # (round, window, block) segment lengths uniform across cores.
    seg_len = np.zeros((R, NWIN, BLOCKS), np.int64)
    for c in range(N_CORES):
        blocks, srcs, dls, scs, rounds = core_arrays[c]
        spans = core_spans[c]
        for (r, wi), idx in spans.items():
            bc = np.bincount(blocks[idx], minlength=BLOCKS)
            seg_len[r, wi] = np.maximum(seg_len[r, wi], bc)
    # 128-align every block segment (PE matmul PSUM outputs must start at
    # partition 0), so each 128-chunk holds a single block
    seg_len = ((seg_len + 127) // 128) * 128
    span_len = seg_len.sum(axis=2)

    # stream offsets
    S = int(span_len.sum())
    span_off = {}
    off = 0
    for r in range(R):
        for wi in range(NWIN):
            span_off[(r, wi)] = off
            off += int(span_len[r, wi])
    assert off == S

    # gather calls: per (r, w) span, split at GCALL
    gather_calls = []  # (w, start, n)
    for r in range(R):
        for wi in range(NWIN):
            L = int(span_len[r, wi])
            o = span_off[(r, wi)]
            p = 0
            while p < L:
                n = min(GCALL, L - p)
                gather_calls.append((wi, o + p, n))
                p += n
    # scatter calls: per round, split at SCALL
    scatter_calls = []  # (round, start, n)
    for r in range(R):
        o = span_off[(r, 0)]
        L = int(span_len[r].sum())
        p = 0
        while p < L:
            n = min(SCALL, L - p)
            scatter_calls.append((r, o + p, n))
            p += n

    # chunk block-segment table: for each 128-chunk: list of (block, off, len)
    chunk_segs = []
    for r in range(R):
        for wi in range(NWIN):
            L = int(span_len[r, wi])
            if L == 0:
                continue
            segs = []
            pos = 0
            for b in range(BLOCKS):
                n = int(seg_len[r, wi, b])
                if n:
                    segs.append((b, pos, n))
                    pos += n
            # walk chunks
            for cstart in range(0, L, 128):
                cs = []
                for b, p0, n in segs:
                    a = max(p0, cstart)
                    bnd = min(p0 + n, cstart + 128)
                    if bnd > a:
                        cs.append((b, a - cstart, bnd - a))
                chunk_segs.append(cs)

    layout = {
        "R": R,
        "S": S,
        "span_len": span_len,
        "span_off": span_off,
        "seg_len": seg_len,
        "gather_calls": gather_calls,
        "scatter_calls": scatter_calls,
        "chunk_segs": chunk_segs,
    }

    # per-core planes
    per_core = []
    for c in range(N_CORES):
        blocks, srcs, dls, scs, rounds = core_arrays[c]
        spans = core_spans[c]
        gidx = np.zeros(S, np.int16)
        sidx = np.zeros(S, np.int16)
        scale = np.zeros(S, np.float32)
        # defaults for pads: gidx 0 (valid), dest trash, scale 0
        pos_all = np.arange(S)
        sidx[:] = (SHARD + (pos_all % 128)).astype(np.int16)  # acc0 trash
        for r in range(R):
            for wi in range(NWIN):
                o = span_off[(r, wi)]
                idx = spans.get((r, wi))
                if idx is None or len(idx) == 0:
                    continue
                # place entries honoring uniform block segments
                p0 = 0
                bc = np.bincount(blocks[idx], minlength=BLOCKS)
                bstart = np.concatenate([[0], np.cumsum(bc)[:-1]])
                # idx is sorted by block already
                for b in range(BLOCKS):
                    n = int(bc[b])
                    if n == 0:
                        p0 += int(seg_len[r, wi, b])
                        continue
                    sel = idx[bstart[b]: bstart[b] + n]
                    sl = slice(o + p0, o + p0 + n)
                    gidx[sl] = (srcs[sel] - wi * WIN).astype(np.int16)
                    sidx[sl] = dls[sel].astype(np.int16)
                    scale[sl] = scs[sel]
                    p0 += int(seg_len[r, wi, b])
        per_core.append({
            "gidx": _idx_plane16(gidx, S),
            "sidx": _idx_plane16(sidx, S),
            "scale": _plane128(scale, S, np.float32),
        })
    return layout, per_core


# --------------------------------------------------------------------------
# Device program
# --------------------------------------------------------------------------

def _build(layout):
    bf16 = mybir.dt.bfloat16
    f32 = mybir.dt.float32
    i16 = mybir.dt.int16
    S = layout["S"]

    nc = bacc.Bacc(dynamic_dma_scratch_size=SCRATCH)
    xb_d = nc.declare_dram_parameter("xb", [N_NODES, D], bf16, isOutput=False)
    xs_d = nc.declare_dram_parameter("xs", [SHARD, D], bf16, isOutput=False)
    gidx_d = nc.declare_dram_parameter("gidx", [P, S // 16], i16, isOutput=False)
    sidx_d = nc.declare_dram_parameter("sidx", [P, S // 16], i16, isOutput=False)
    scale_d = nc.declare_dram_parameter("scale", [P, S // 128], f32, isOutput=False)
    ab_d = nc.declare_dram_parameter("Ab", [BLOCKS * D, D], bf16, isOutput=False)
    cwT_d = nc.declare_dram_parameter("CwT", [D, D], bf16, isOutput=False)
    cb_d = nc.declare_dram_parameter("Cb", [1, D], bf16, isOutput=False)
    out_d = nc.declare_dram_parameter("out", [OUT_ROWS, D], f32, isOutput=True)

    from contextlib import ExitStack

    with tile.TileContext(nc) as tc, ExitStack() as ctx:
        cpool = ctx.enter_context(tc.tile_pool(name="const", bufs=1))
        gpool = ctx.enter_context(tc.tile_pool(name="gath", bufs=26))
        xtpool = ctx.enter_context(tc.tile_pool(name="xT", bufs=3))
        scpool = ctx.enter_context(tc.tile_pool(name="scat", bufs=2))
        rpool = ctx.enter_context(tc.tile_pool(name="resid", bufs=3))
        pt_pool = ctx.enter_context(tc.tile_pool(name="psT", bufs=2, space="PSUM"))
        pm_pool = ctx.enter_context(tc.tile_pool(name="psM", bufs=3, space="PSUM"))

        # ---- constants -------------------------------------------------
        nc.gpsimd.load_library(mlp)
        ident = cpool.tile([P, P], bf16, tag="ident")
        make_identity(nc, ident[:, :])
        ones = cpool.tile([1, P], bf16, tag="ones")
        nc.gpsimd.memset(ones[:, :], 1.0)
        cb_t = cpool.tile([1, D], bf16, tag="cb")
        nc.sync.dma_start(out=cb_t[:, :], in_=cb_d[:, :])
        cwT_t = cpool.tile([D, D], bf16, tag="cwT")
        nc.sync.dma_start(out=cwT_t[:, :], in_=cwT_d[:, :])
        a_tiles = []
        for b in range(BLOCKS):
            at = cpool.tile([D, D], bf16, tag=f"A{b}")
            nc.sync.dma_start(out=at[:, :], in_=ab_d[b * D:(b + 1) * D, :])
            a_tiles.append(at)

        # ---- index planes ----------------------------------------------
        gidx_t = cpool.tile([P, S // 16], i16, tag="gidx")
        nc.sync.dma_start(out=gidx_t[:, :], in_=gidx_d[:, :])
        sidx_t = cpool.tile([P, S // 16], i16, tag="sidx")
        nc.sync.dma_start(out=sidx_t[:, :], in_=sidx_d[:, :])
        scale_t = cpool.tile([P, S // 128], f32, tag="scale")
        nc.sync.dma_start(out=scale_t[:, :], in_=scale_d[:, :])

        # ---- residual phase --------------------------------------------
        resid_writes = []
        n_rw = (SHARD + P - 1) // P
        for j in range(n_rw):
            rows = min(P, SHARD - j * P)
            xt = rpool.tile([P, D], bf16, tag="rx")
            nc.sync.dma_start(out=xt[:rows, :], in_=xs_d[j * P:j * P + rows, :])
            ps_t = pt_pool.tile([P, P], bf16, tag="psT")
            nc.tensor.transpose(out=ps_t[:, :rows], in_=xt[:rows, :],
                                identity=ident[:rows, :rows])
            xT = rpool.tile([P, P], bf16, tag="rxT")
            nc.vector.tensor_copy(out=xT[:, :rows], in_=ps_t[:, :rows])
            ps_o = pm_pool.tile([P, STRIP * P], f32, tag="pm")
            nc.tensor.matmul(out=ps_o[:rows, :D], lhsT=xT[:, :rows],
                             rhs=cwT_t[:, :], start=True, stop=False)
            nc.tensor.matmul(out=ps_o[:rows, :D], lhsT=ones[:1, :rows],
                             rhs=cb_t[:1, :], start=False, stop=True)
            ot = rpool.tile([P, D], f32, tag="rout")
            nc.scalar.copy(out=ot[:rows, :], in_=ps_o[:rows, :D])
            wi = nc.sync.dma_start(out=out_d[j * P:j * P + rows, :],
                                   in_=ot[:rows, :])
            resid_writes.append(wi)

        joiner0 = cpool.tile([1, 8], f32, tag="join0")
        prev_join = nc.gpsimd.memset(joiner0[:1, :], 0.0)
        for wi in resid_writes:
            add_dep_helper(prev_join.ins, wi.ins, sync=True,
                           reason="join residual writes")

        # ---- edge pipeline ---------------------------------------------
        R = layout["R"]
        span_len = layout["span_len"]
        span_off = layout["span_off"]
        chunk_segs = layout["chunk_segs"]
        gather_calls = layout["gather_calls"]
        scatter_calls = layout["scatter_calls"]

        # group tables per round for issue order
        g_by_round = {}
        gi = 0
        for r in range(R):
            lst = []
            for wi in range(NWIN):
                L = int(span_len[r, wi])
                p = 0
                while p < L:
                    w2, start, n = gather_calls[gi]
                    assert w2 == wi and start == span_off[(r, wi)] + p
                    lst.append((wi, start, n))
                    gi += 1
                    p += n
            g_by_round[r] = lst
        assert gi == len(gather_calls)
        s_by_round = {}
        for r, start, n in scatter_calls:
            s_by_round.setdefault(r, []).append((start, n))

        chunk_ptr = 0  # index into chunk_segs (stream order)
        prev_scatters = None

        def issue_gathers(r):
            gtiles = []
            for wi, start, n in g_by_round[r]:
                gt = gpool.tile([P, GCALL // 128, P], bf16, tag="g")
                cols = n // 128
                nc.gpsimd.dma_gather(
                    gt[:, :cols, :],
                    xb_d[wi * WIN: min(N_NODES, (wi + 1) * WIN), :],
                    gidx_t[:, start // 16: (start + n) // 16],
                    n, n, D,
                )
                gtiles.append((start, n, gt))
            return gtiles

        pending_g = {0: issue_gathers(0)}
        for r in range(R):
            round_insts = []
            gtiles = pending_g.pop(r)
            # lookahead: desc-gen the NEXT round's gathers before this
            # round's join stalls the Pool on the previous drain
            if r + 1 < R:
                pending_g[r + 1] = issue_gathers(r + 1)
            if prev_scatters is not None:
                jt = cpool.tile([1, 8], f32, tag="join")
                ji = nc.gpsimd.memset(jt[:1, :], 0.0)
                for si in prev_scatters:
                    add_dep_helper(ji.ins, si.ins, sync=True,
                                   reason="join round scatters")
                prev_join = ji

            # compute + scatter per SCALL strip
            for s_start, s_n in s_by_round[r]:
                sc_t = scpool.tile([P, SCALL // 128, P], f32, tag="sc")
                s_cols = s_n // 128
                for grp in range(0, s_cols, STRIP):
                    gcols = min(STRIP, s_cols - grp)
                    pm = pm_pool.tile([P, STRIP * P], f32, tag="pm")
                    for jj in range(gcols):
                        pos = s_start + (grp + jj) * 128
                        # locate gather tile
                        gt = None
                        for a, n, t0 in gtiles:
                            if a <= pos < a + n:
                                gt = (t0, (pos - a) // 128)
                                break
                        assert gt is not None
                        t0, gcol = gt
                        ps_t = pt_pool.tile([P, P], bf16, tag="psT")
                        nc.tensor.transpose(out=ps_t[:, :],
                                            in_=t0[:, gcol, :],
                                            identity=ident[:, :])
                        xT = xtpool.tile([P, P], bf16, tag="xT")
                        nc.vector.tensor_copy(out=xT[:, :], in_=ps_t[:, :])
                        segs = chunk_segs[chunk_ptr]
                        chunk_ptr += 1
                        assert len(segs) == 1 and segs[0][1] == 0 \
                            and segs[0][2] == 128, segs
                        b = segs[0][0]
                        nc.tensor.matmul(
                            out=pm[:, jj * P:(jj + 1) * P],
                            lhsT=xT[:, :],
                            rhs=a_tiles[b][:, :],
                            start=True, stop=True)
                        # scale-copy psum chunk -> scatter strip (per chunk,
                        # per-partition scalar)
                        col = pos // 128
                        if jj % 2 == 0:
                            nc.vector.tensor_scalar_mul(
                                out=sc_t[:, grp + jj, :],
                                in0=pm[:, jj * P:(jj + 1) * P],
                                scalar1=scale_t[:, col:col + 1])
                        else:
                            nc.scalar.mul(
                                out=sc_t[:, grp + jj, :],
                                in_=pm[:, jj * P:(jj + 1) * P],
                                mul=scale_t[:, col:col + 1])
                si = nc.gpsimd.dma_scatter_add(
                    out_d[:, :],
                    sc_t[:, :s_cols, :],
                    sidx_t[:, s_start // 16: (s_start + s_n) // 16],
                    s_n, s_n, D,
                )
                add_dep_helper(si.ins, prev_join.ins, sync=True,
                               reason="round gate")
                round_insts.append(si)
            prev_scatters = round_insts
        assert chunk_ptr == len(chunk_segs)

        # ---- merge accumulator 1 into the output rows -------------------
        fj = cpool.tile([1, 8], f32, tag="joinF")
        fji = nc.gpsimd.memset(fj[:1, :], 0.0)
        for si in (prev_scatters or []):
            add_dep_helper(fji.ins, si.ins, sync=True,
                           reason="final scatter join")
        for j in range(n_rw):
            rows = min(P, SHARD - j * P)
            t0 = rpool.tile([P, D], f32, tag="m0")
            l0 = nc.sync.dma_start(out=t0[:rows, :],
                                   in_=out_d[j * P:j * P + rows, :])
            add_dep_helper(l0.ins, fji.ins, sync=True, reason="merge gate")
            t1 = rpool.tile([P, D], f32, tag="m1")
            l1 = nc.sync.dma_start(
                out=t1[:rows, :],
                in_=out_d[ACC_STRIDE + j * P: ACC_STRIDE + j * P + rows, :])
            add_dep_helper(l1.ins, fji.ins, sync=True, reason="merge gate")
            t2 = rpool.tile([P, D], f32, tag="m2")
            nc.vector.tensor_add(out=t2[:rows, :], in0=t0[:rows, :],
                                 in1=t1[:rows, :])
            nc.sync.dma_start(out=out_d[j * P:j * P + rows, :],
                              in_=t2[:rows, :])

    nc.compile()
    return nc


# --------------------------------------------------------------------------
# Entry point
# --------------------------------------------------------------------------

_CACHE = {}


def kernel(x, edge_idx_r1, edge_idx_r2, edge_idx_r3, A_r1, A_r2, A_r3, C_w, C_b):
    global LAST_RESULTS
    x = np.asarray(x, np.float32)
    assert x.shape == (N_NODES, D)
    edges = [(edge_idx_r1, 1), (edge_idx_r2, 2), (edge_idx_r3, 3)]
    layout, per_core = _prep(edges)

    key = (layout["S"], layout["R"],
           tuple(layout["gather_calls"]), tuple(layout["scatter_calls"]))
    if key not in _CACHE:
        _CACHE[key] = _build(layout)
    nc = _CACHE[key]

    xb = np.ascontiguousarray(x.astype(ml_dtypes.bfloat16))
    # A blocks stacked: A1 | A2[0:128] | A2[128:] | A3[0:128] | A3[128:256] | A3[256:]
    ab = np.concatenate([
        np.asarray(A_r1, np.float32),
        np.asarray(A_r2, np.float32),
        np.asarray(A_r3, np.float32),
    ], axis=0).astype(ml_dtypes.bfloat16)
    cwT = np.ascontiguousarray(np.asarray(C_w, np.float32).T).astype(
        ml_dtypes.bfloat16)
    cb = np.asarray(C_b, np.float32).reshape(1, D).astype(ml_dtypes.bfloat16)

    in_maps = []
    for c in range(N_CORES):
        in_maps.append({
            "xb": xb,
            "xs": np.ascontiguousarray(xb[c * SHARD:(c + 1) * SHARD]),
            "gidx": per_core[c]["gidx"],
            "sidx": per_core[c]["sidx"],
            "scale": per_core[c]["scale"],
            "Ab": ab,
            "CwT": cwT,
            "Cb": cb,
        })

    res = run_bass_kernel_spmd(nc, in_maps, list(range(N_CORES)), trace=TRACE)
    LAST_RESULTS = res
    return np.concatenate([r["out"][:SHARD] for r in res.results], axis=0)



# revision 6
# speedup vs baseline: 1.5220x; 1.5220x over previous
"""HGNN layer kernel for 8 Trainium2 NeuronCores — gather + on-chip
segmented-sum version (no dma_scatter_add).

out = x @ C_w.T + C_b + sum_t scatter_add(dest_t, (1/counts_t[dest]) * msg_t)
msg_t[g] = concat(x[src_{g,k}] for k in arity) @ A_t

Why this structure: the previous kernel was bottlenecked by SWDGE
descriptor generation on the gpsimd engine (~8.6 ns/descriptor, serialized)
with gathers AND scatter-adds both paying per-entry descriptors.  This
version removes the scatter side entirely:

  - Per-entry decomposition: agg[d] = sum_b (sum_{e: dest=d, blk=b}
    scale_e * x[src_e]) @ A_b.  The inner weighted sum (h_b) is computed
    ON CHIP with PE matmuls against scaled one-hot "segment" matrices
    built by the DVE (tensor_scalar is_equal*mult against an iota tile),
    accumulating in PSUM per (dest-tile, block).
  - agg tile = sum_b h_b @ A_b, computed with accumulating matmuls that
    also fold in the residual (x @ C_w.T via a host-pre-transposed x
    shard) and the bias (rank-1 ones @ C_b).  Output rows are written
    once, contiguously.  No CCE read-modify-write, no occurrence rounds.
  - Gathers remain SWDGE but with a single index window: x is uploaded
    as xq = x.reshape(25000, 512) bf16 (4 nodes per 1024B row), idx =
    src//4 < 25000 fits int16, and the entry picks its node via the
    src%4 slot slice of the gathered row at matmul time.

Entry stream layout (uniform across cores as SPMD requires): entries
sorted by (tau=dest_local//128, block, slot=src%4, src); per-(tau, block,
slot) run lengths are max'ed over cores; pad entries get idx 0 and
rank -1 (the is_equal one-hot zeroes them out).
"""

import sys

for _p in ("/opt/trn_rl_repo",):
    if _p not in sys.path:
        sys.path.insert(0, _p)

import numpy as np
import ml_dtypes

import concourse.bass as bass
import concourse.bacc as bacc
import concourse.mybir as mybir
import concourse.tile as tile
from concourse.bass_utils import run_bass_kernel_spmd
from concourse.library_config import mlp

N_CORES = 8
D = 128
P = 128
N_NODES = 100000
SHARD = N_NODES // N_CORES   # 12500
NTAU = (SHARD + P - 1) // P  # 98
QROWS = N_NODES // 4         # 25000 packed rows of 4 nodes
QELEM = 4 * D                # 512 bf16 elems = 1024B per packed row
SCRATCH = 32768              # dynamic dma scratch bytes/partition (ring = /16)
GCALL = 2048                 # gather idxs per call (= ring capacity)
BLOCKS = 6                   # (t0,k0) (t1,k0) (t1,k1) (t2,k0) (t2,k1) (t2,k2)
NSLOT = 4

TRACE = False
LAST_RESULTS = None


# --------------------------------------------------------------------------
# Host-side preparation
# --------------------------------------------------------------------------

def _idx_plane16(vals, length):
    """int16 plane [128, length//16]: entry i -> [i%16, i//16], replicated
    across the 8 gpsimd cores."""
    assert length % 16 == 0
    v = np.zeros(length, np.int16)
    v[: len(vals)] = vals
    pl = v.reshape(length // 16, 16).T  # [16, C]
    return np.ascontiguousarray(np.tile(pl, (8, 1)))


def _prep(edges):
    """Build the uniform entry-stream layout.

    Returns (layout, per_core):
      layout: S128, run table, per-chunk op lists, per-(tau) completion info
      per_core: gidx plane, rank plane, scale plane
    """
    # per-core entry arrays
    core_tau = [[] for _ in range(N_CORES)]
    core_rank = [[] for _ in range(N_CORES)]
    core_b = [[] for _ in range(N_CORES)]
    core_s = [[] for _ in range(N_CORES)]
    core_q = [[] for _ in range(N_CORES)]
    core_sc = [[] for _ in range(N_CORES)]
    for t, (e, arity) in enumerate(edges):
        e = np.asarray(e)
        g = e.shape[1] // arity
        dest = e[1].reshape(g, arity)[:, 0].astype(np.int64)
        srcs = e[0].reshape(g, arity).astype(np.int64)
        counts = np.bincount(dest, minlength=N_NODES)
        inv = np.zeros(N_NODES, np.float32)
        nz = counts > 0
        inv[nz] = np.float32(1.0) / counts[nz].astype(np.float32)
        blk0 = {0: 0, 1: 1, 2: 3}[t]
        core_of = dest // SHARD
        dl = dest - core_of * SHARD
        sc = inv[dest]
        for c in range(N_CORES):
            sel = np.where(core_of == c)[0]
            for k in range(arity):
                s_k = srcs[sel, k]
                core_tau[c].append(dl[sel] // P)
                core_rank[c].append(dl[sel] % P)
                core_b[c].append(np.full(len(sel), blk0 + k, np.int64))
                core_s[c].append(s_k % NSLOT)
                core_q[c].append(s_k // NSLOT)
                core_sc[c].append(sc[sel])

    cores = []
    NRUN = NTAU * BLOCKS * NSLOT
    seg_len = np.zeros(NRUN, np.int64)
    for c in range(N_CORES):
        tau = np.concatenate(core_tau[c])
        rank = np.concatenate(core_rank[c])
        b = np.concatenate(core_b[c])
        s = np.concatenate(core_s[c])
        q = np.concatenate(core_q[c])
        sc = np.concatenate(core_sc[c])
        run = (tau * BLOCKS + b) * NSLOT + s
        order = np.lexsort((q, run))
        run, rank, q, sc = run[order], rank[order], q[order], sc[order]
        np.maximum(seg_len, np.bincount(run, minlength=NRUN), out=seg_len)
        cores.append((run, rank, q, sc))

    run_start = np.concatenate([[0], np.cumsum(seg_len)])
    S = int(run_start[-1])
    S128 = (S + 127) // 128 * 128

    # ops: per run, one op per chunk it overlaps
    #   op fields: (chunk, tau, b, slot, start, stop)
    run_j0 = run_start[:-1] // 128               # first chunk of run
    run_j1 = (run_start[:-1] + np.maximum(seg_len, 1) - 1) // 128  # last chunk
    op_base = np.zeros(NRUN, np.int64)
    ops = []            # list of (chunk, tau, b, slot, start, stop)
    # group boundaries: group = (tau, b); first/last op across its runs
    n_ops_of_group = {}
    for r in range(NRUN):
        if seg_len[r] == 0:
            continue
        op_base[r] = len(ops)
        tau, rem = divmod(r, BLOCKS * NSLOT)
        b, s = divmod(rem, NSLOT)
        for j in range(int(run_j0[r]), int(run_j1[r]) + 1):
            ops.append([j, tau, b, s, False, False])
            n_ops_of_group.setdefault((tau, b), []).append(len(ops) - 1)
    for (tau, b), idxs in n_ops_of_group.items():
        ops[idxs[0]][4] = True
        ops[idxs[-1]][5] = True
    n_ops = len(ops)

    # per-chunk op lists and per-chunk tau completions
    nchunks = S128 // 128
    chunk_ops = [[] for _ in range(nchunks)]
    for i, op in enumerate(ops):
        chunk_ops[op[0]].append(i)
    tau_last_chunk = np.full(NTAU, -1, np.int64)
    for i, (j, tau, b, s, st, sp) in enumerate(ops):
        tau_last_chunk[tau] = max(tau_last_chunk[tau], j)
    assert (tau_last_chunk >= 0).all(), "every dest tile must have entries"
    tau_done_at = [[] for _ in range(nchunks)]
    for tau in range(NTAU):
        if tau_last_chunk[tau] >= 0:
            tau_done_at[int(tau_last_chunk[tau])].append(tau)
    tau_blocks = {}
    for tau in range(NTAU):
        tau_blocks[tau] = sorted(
            b for (t2, b) in n_ops_of_group.keys() if t2 == tau)

    layout = {
        "S128": S128,
        "n_ops": n_ops,
        "ops": ops,
        "chunk_ops": chunk_ops,
        "tau_done_at": tau_done_at,
        "tau_blocks": tau_blocks,
    }

    # per-core planes
    per_core = []
    for c in range(N_CORES):
        run, rank, q, sc = cores[c]
        # position within run (entries already sorted by run)
        first = np.searchsorted(run, run, side="left")
        posin = np.arange(len(run)) - first
        pos = run_start[run] + posin
        gidx = np.zeros(S128, np.int16)
        gidx[pos] = q.astype(np.int16)
        opid = op_base[run] + pos // 128 - run_j0[run]
        rank_pl = np.full((P, n_ops), -1.0, np.float32)
        scale_pl = np.zeros((P, n_ops), np.float32)
        rank_pl[pos % 128, opid] = rank.astype(np.float32)
        scale_pl[pos % 128, opid] = sc
        per_core.append({
            "gidx": _idx_plane16(gidx, S128),
            "rankp": np.ascontiguousarray(rank_pl),
            "scalep": np.ascontiguousarray(scale_pl),
        })
    return layout, per_core


# --------------------------------------------------------------------------
# Device program
# --------------------------------------------------------------------------

def _build(layout):
    bf16 = mybir.dt.bfloat16
    f32 = mybir.dt.float32
    i16 = mybir.dt.int16
    S128 = layout["S128"]
    n_ops = layout["n_ops"]
    ops = layout["ops"]
    chunk_ops = layout["chunk_ops"]
    tau_done_at = layout["tau_done_at"]
    tau_blocks = layout["tau_blocks"]
    nchunks = S128 // 128

    nc = bacc.Bacc(dynamic_dma_scratch_size=SCRATCH)
    xq_d = nc.declare_dram_parameter("xq", [QROWS, QELEM], bf16, isOutput=False)
    xT_d = nc.declare_dram_parameter("xT", [P, SHARD], bf16, isOutput=False)
    gidx_d = nc.declare_dram_parameter("gidx", [P, S128 // 16], i16, isOutput=False)
    rank_d = nc.declare_dram_parameter("rankp", [P, n_ops], f32, isOutput=False)
    scale_d = nc.declare_dram_parameter("scalep", [P, n_ops], f32, isOutput=False)
    iota_d = nc.declare_dram_parameter("iota", [P, P], f32, isOutput=False)
    ab_d = nc.declare_dram_parameter("Ab", [BLOCKS * D, D], bf16, isOutput=False)
    cwT_d = nc.declare_dram_parameter("CwT", [D, D], bf16, isOutput=False)
    cb_d = nc.declare_dram_parameter("Cb", [1, D], bf16, isOutput=False)
    out_d = nc.declare_dram_parameter("out", [SHARD, D], f32, isOutput=True)

    from contextlib import ExitStack

    with tile.TileContext(nc) as tc, ExitStack() as ctx:
        cpool = ctx.enter_context(tc.tile_pool(name="const", bufs=1))
        gpool = ctx.enter_context(tc.tile_pool(name="gath", bufs=3))
        segpool = ctx.enter_context(tc.tile_pool(name="seg", bufs=8))
        hspool = ctx.enter_context(tc.tile_pool(name="hsb", bufs=2))
        opool = ctx.enter_context(tc.tile_pool(name="outb", bufs=3))
        hpsum = ctx.enter_context(tc.tile_pool(name="hps", bufs=1, space="PSUM"))
        opsum = ctx.enter_context(tc.tile_pool(name="ops", bufs=2, space="PSUM"))

        # ---- constants -------------------------------------------------
        nc.gpsimd.load_library(mlp)
        ones = cpool.tile([1, P], bf16, tag="ones")
        nc.vector.memset(ones[:, :], 1.0)
        iota_t = cpool.tile([P, P], f32, tag="iota")
        nc.sync.dma_start(out=iota_t[:, :], in_=iota_d[:, :])
        cb_t = cpool.tile([1, D], bf16, tag="cb")
        nc.sync.dma_start(out=cb_t[:, :], in_=cb_d[:, :])
        cwT_t = cpool.tile([D, D], bf16, tag="cwT")
        nc.sync.dma_start(out=cwT_t[:, :], in_=cwT_d[:, :])
        xT_t = cpool.tile([P, SHARD], bf16, tag="xT")
        nc.sync.dma_start(out=xT_t[:, :], in_=xT_d[:, :])
        a_tiles = []
        for b in range(BLOCKS):
            at = cpool.tile([D, D], bf16, tag=f"A{b}")
            nc.sync.dma_start(out=at[:, :], in_=ab_d[b * D:(b + 1) * D, :])
            a_tiles.append(at)
        gidx_t = cpool.tile([P, S128 // 16], i16, tag="gidx")
        nc.sync.dma_start(out=gidx_t[:, :], in_=gidx_d[:, :])
        rank_t = cpool.tile([P, n_ops], f32, tag="rankp")
        nc.sync.dma_start(out=rank_t[:, :], in_=rank_d[:, :])
        scale_t = cpool.tile([P, n_ops], f32, tag="scalep")
        nc.sync.dma_start(out=scale_t[:, :], in_=scale_d[:, :])

        # ---- pipeline --------------------------------------------------
        ncalls = (S128 + GCALL - 1) // GCALL
        cols_per_call = GCALL // 128
        gtiles = [None] * ncalls

        def issue_gather(k):
            start = k * GCALL
            n = min(GCALL, S128 - start)
            gt = gpool.tile([P, cols_per_call, QELEM], bf16, tag="g")
            nc.gpsimd.dma_gather(
                gt[:, : n // 128, :],
                xq_d[:, :],
                gidx_t[:, start // 16: (start + n) // 16],
                n, n, QELEM,
                single_packet=False,
            )
            gtiles[k] = gt

        h_tiles = {}  # (tau, b) -> psum tile

        for k in range(min(3, ncalls)):
            issue_gather(k)

        for k in range(ncalls):
            j0 = k * cols_per_call
            j1 = min(nchunks, j0 + cols_per_call)
            for j in range(j0, j1):
                gt = gtiles[k]
                col = j - j0
                for oi in chunk_ops[j]:
                    _, tau, b, s, o_start, o_stop = ops[oi]
                    seg = segpool.tile([P, P], bf16, tag="seg")
                    nc.vector.tensor_scalar(
                        out=seg[:, :],
                        in0=iota_t[:, :],
                        scalar1=rank_t[:, oi:oi + 1],
                        scalar2=scale_t[:, oi:oi + 1],
                        op0=mybir.AluOpType.is_equal,
                        op1=mybir.AluOpType.mult,
                    )
                    if o_start:
                        h_tiles[(tau, b)] = hpsum.tile(
                            [P, P], f32, name=f"h{b}", tag=f"h{b}")
                    nc.tensor.matmul(
                        out=h_tiles[(tau, b)][:, :],
                        lhsT=gt[:, col, s * D:(s + 1) * D],
                        rhs=seg[:, :],
                        start=o_start, stop=o_stop,
                    )
                # finished dest tiles: second stage
                for tau in tau_done_at[j]:
                    rows = min(P, SHARD - tau * P)
                    po = opsum.tile([P, D], f32, tag="po")
                    blocks = tau_blocks[tau]
                    hs_list = []
                    for b in blocks:
                        hs = hspool.tile([P, P], bf16, tag=f"hs{b}")
                        nc.scalar.copy(out=hs[:, :], in_=h_tiles.pop((tau, b))[:, :])
                        hs_list.append((b, hs))
                    for i, (b, hs) in enumerate(hs_list):
                        nc.tensor.matmul(
                            out=po[:, :],
                            lhsT=hs[:, :],
                            rhs=a_tiles[b][:, :],
                            start=(i == 0), stop=False,
                        )
                    nc.tensor.matmul(
                        out=po[:rows, :],
                        lhsT=xT_t[:, tau * P: tau * P + rows],
                        rhs=cwT_t[:, :],
                        start=(len(hs_list) == 0), stop=False,
                    )
                    nc.tensor.matmul(
                        out=po[:rows, :],
                        lhsT=ones[:1, :rows],
                        rhs=cb_t[:1, :],
                        start=False, stop=True,
                    )
                    ot = opool.tile([P, D], f32, tag="o")
                    nc.vector.tensor_copy(out=ot[:rows, :], in_=po[:rows, :])
                    nc.sync.dma_start(
                        out=out_d[tau * P: tau * P + rows, :],
                        in_=ot[:rows, :])
            if k + 3 < ncalls:
                issue_gather(k + 3)

    nc.compile()
    return nc


# --------------------------------------------------------------------------
# Entry point
# --------------------------------------------------------------------------

_CACHE = {}


def kernel(x, edge_idx_r1, edge_idx_r2, edge_idx_r3, A_r1, A_r2, A_r3, C_w, C_b):
    global LAST_RESULTS
    x = np.asarray(x, np.float32)
    assert x.shape == (N_NODES, D)
    edges = [(edge_idx_r1, 1), (edge_idx_r2, 2), (edge_idx_r3, 3)]
    layout, per_core = _prep(edges)

    key = (layout["S128"], layout["n_ops"],
           tuple(tuple(o) for o in layout["ops"]))
    if key not in _CACHE:
        _CACHE[key] = _build(layout)
    nc = _CACHE[key]

    xb = x.astype(ml_dtypes.bfloat16)
    xq = np.ascontiguousarray(xb.reshape(QROWS, QELEM))
    iota = np.ascontiguousarray(
        np.broadcast_to(np.arange(P, dtype=np.float32), (P, P)))
    ab = np.concatenate([
        np.asarray(A_r1, np.float32),
        np.asarray(A_r2, np.float32),
        np.asarray(A_r3, np.float32),
    ], axis=0).astype(ml_dtypes.bfloat16)
    cwT = np.ascontiguousarray(np.asarray(C_w, np.float32).T).astype(
        ml_dtypes.bfloat16)
    cb = np.asarray(C_b, np.float32).reshape(1, D).astype(ml_dtypes.bfloat16)

    in_maps = []
    for c in range(N_CORES):
        in_maps.append({
            "xq": xq,
            "xT": np.ascontiguousarray(xb[c * SHARD:(c + 1) * SHARD].T),
            "gidx": per_core[c]["gidx"],
            "rankp": per_core[c]["rankp"],
            "scalep": per_core[c]["scalep"],
            "iota": iota,
            "Ab": ab,
            "CwT": cwT,
            "Cb": cb,
        })

    res = run_bass_kernel_spmd(nc, in_maps, list(range(N_CORES)), trace=TRACE)
    LAST_RESULTS = res
    return np.concatenate([r["out"] for r in res.results], axis=0)


# revision 14
# speedup vs baseline: 2.4767x; 1.6272x over previous
"""HGNN layer kernel for 8 Trainium2 NeuronCores — gather + on-chip
segmented-sum version (no dma_scatter_add).

out = x @ C_w.T + C_b + sum_t scatter_add(dest_t, (1/counts_t[dest]) * msg_t)
msg_t[g] = concat(x[src_{g,k}] for k in arity) @ A_t

Why this structure: the previous kernel was bottlenecked by SWDGE
descriptor generation on the gpsimd engine (~8.6 ns/descriptor, serialized)
with gathers AND scatter-adds both paying per-entry descriptors.  This
version removes the scatter side entirely:

  - Per-entry decomposition: agg[d] = sum_b (sum_{e: dest=d, blk=b}
    scale_e * x[src_e]) @ A_b.  The inner weighted sum (h_b) is computed
    ON CHIP with PE matmuls against scaled one-hot "segment" matrices
    built by the DVE (tensor_scalar is_equal*mult against an iota tile),
    accumulating in PSUM per (dest-tile, block).
  - agg tile = sum_b h_b @ A_b, computed with accumulating matmuls that
    also fold in the residual (x @ C_w.T via a host-pre-transposed x
    shard) and the bias (rank-1 ones @ C_b).  Output rows are written
    once, contiguously.  No CCE read-modify-write, no occurrence rounds.
  - Gathers remain SWDGE but with a single index window: x is uploaded
    as xq = x.reshape(25000, 512) bf16 (4 nodes per 1024B row), idx =
    src//4 < 25000 fits int16, and the entry picks its node via the
    src%4 slot slice of the gathered row at matmul time.

Entry stream layout (uniform across cores as SPMD requires): entries
sorted by (tau=dest_local//128, block, slot=src%4, src); per-(tau, block,
slot) run lengths are max'ed over cores; pad entries get idx 0 and
rank -1 (the is_equal one-hot zeroes them out).
"""

import sys

for _p in ("/opt/trn_rl_repo",):
    if _p not in sys.path:
        sys.path.insert(0, _p)

import numpy as np
import ml_dtypes

import concourse.bass as bass
import concourse.bacc as bacc
import concourse.mybir as mybir
import concourse.tile as tile
from concourse.bass_utils import run_bass_kernel_spmd
from concourse.library_config import mlp

N_CORES = 8
D = 128
P = 128
N_NODES = 100000
SHARD = N_NODES // N_CORES   # 12500
NTAU = (SHARD + P - 1) // P  # 98
QROWS = N_NODES // 4         # 25000 packed rows of 4 nodes
QELEM = 4 * D                # 512 bf16 elems = 1024B per packed row
SCRATCH = 16384              # dynamic dma scratch bytes/partition (ring = /16)
GCALL = 1024                 # gather idxs per call (= ring capacity)
SEGBLK = 32                  # seg matrices per DMA load
BLOCKS = 6                   # (t0,k0) (t1,k0) (t1,k1) (t2,k0) (t2,k1) (t2,k2)
NSLOT = 4

TRACE = False
LAST_RESULTS = None


# --------------------------------------------------------------------------
# Host-side preparation
# --------------------------------------------------------------------------

def _idx_plane16(vals, length):
    """int16 plane [128, length//16]: entry i -> [i%16, i//16], replicated
    across the 8 gpsimd cores."""
    assert length % 16 == 0
    v = np.zeros(length, np.int16)
    v[: len(vals)] = vals
    pl = v.reshape(length // 16, 16).T  # [16, C]
    return np.ascontiguousarray(np.tile(pl, (8, 1)))


def _prep(edges):
    """Build the uniform entry-stream layout.

    Returns (layout, per_core):
      layout: S128, run table, per-chunk op lists, per-(tau) completion info
      per_core: gidx plane, rank plane, scale plane
    """
    # per-core entry arrays
    core_tau = [[] for _ in range(N_CORES)]
    core_rank = [[] for _ in range(N_CORES)]
    core_b = [[] for _ in range(N_CORES)]
    core_s = [[] for _ in range(N_CORES)]
    core_q = [[] for _ in range(N_CORES)]
    core_sc = [[] for _ in range(N_CORES)]
    for t, (e, arity) in enumerate(edges):
        e = np.asarray(e)
        g = e.shape[1] // arity
        dest = e[1].reshape(g, arity)[:, 0].astype(np.int64)
        srcs = e[0].reshape(g, arity).astype(np.int64)
        counts = np.bincount(dest, minlength=N_NODES)
        inv = np.zeros(N_NODES, np.float32)
        nz = counts > 0
        inv[nz] = np.float32(1.0) / counts[nz].astype(np.float32)
        blk0 = {0: 0, 1: 1, 2: 3}[t]
        core_of = dest // SHARD
        dl = dest - core_of * SHARD
        sc = inv[dest]
        for c in range(N_CORES):
            sel = np.where(core_of == c)[0]
            for k in range(arity):
                s_k = srcs[sel, k]
                core_tau[c].append(dl[sel] // P)
                core_rank[c].append(dl[sel] % P)
                core_b[c].append(np.full(len(sel), blk0 + k, np.int64))
                core_s[c].append(s_k % NSLOT)
                core_q[c].append(s_k // NSLOT)
                core_sc[c].append(sc[sel])

    cores = []
    NRUN = NTAU * BLOCKS * NSLOT
    seg_len = np.zeros(NRUN, np.int64)
    for c in range(N_CORES):
        tau = np.concatenate(core_tau[c])
        rank = np.concatenate(core_rank[c])
        b = np.concatenate(core_b[c])
        s = np.concatenate(core_s[c])
        q = np.concatenate(core_q[c])
        sc = np.concatenate(core_sc[c])
        run = (tau * BLOCKS + b) * NSLOT + s
        order = np.lexsort((q, run))
        run, rank, q, sc = run[order], rank[order], q[order], sc[order]
        np.maximum(seg_len, np.bincount(run, minlength=NRUN), out=seg_len)
        cores.append((run, rank, q, sc))

    run_start = np.concatenate([[0], np.cumsum(seg_len)])
    S = int(run_start[-1])
    S128 = (S + 127) // 128 * 128

    # ops: per run, one op per chunk it overlaps
    #   op fields: (chunk, tau, b, slot, start, stop)
    run_j0 = run_start[:-1] // 128               # first chunk of run
    run_j1 = (run_start[:-1] + np.maximum(seg_len, 1) - 1) // 128  # last chunk
    op_base = np.zeros(NRUN, np.int64)
    ops = []            # list of (chunk, tau, b, slot, start, stop)
    # group boundaries: group = (tau, b); first/last op across its runs
    n_ops_of_group = {}
    for r in range(NRUN):
        if seg_len[r] == 0:
            continue
        op_base[r] = len(ops)
        tau, rem = divmod(r, BLOCKS * NSLOT)
        b, s = divmod(rem, NSLOT)
        for j in range(int(run_j0[r]), int(run_j1[r]) + 1):
            ops.append([j, tau, b, s, False, False])
            n_ops_of_group.setdefault((tau, b), []).append(len(ops) - 1)
    for (tau, b), idxs in n_ops_of_group.items():
        ops[idxs[0]][4] = True
        ops[idxs[-1]][5] = True
    n_ops = len(ops)

    # per-chunk op lists and per-chunk tau completions
    nchunks = S128 // 128
    chunk_ops = [[] for _ in range(nchunks)]
    for i, op in enumerate(ops):
        chunk_ops[op[0]].append(i)
    tau_last_chunk = np.full(NTAU, -1, np.int64)
    for i, (j, tau, b, s, st, sp) in enumerate(ops):
        tau_last_chunk[tau] = max(tau_last_chunk[tau], j)
    assert (tau_last_chunk >= 0).all(), "every dest tile must have entries"
    tau_done_at = [[] for _ in range(nchunks)]
    for tau in range(NTAU):
        if tau_last_chunk[tau] >= 0:
            tau_done_at[int(tau_last_chunk[tau])].append(tau)
    tau_blocks = {}
    for tau in range(NTAU):
        tau_blocks[tau] = sorted(
            b for (t2, b) in n_ops_of_group.keys() if t2 == tau)

    layout = {
        "S128": S128,
        "n_ops": n_ops,
        "ops": ops,
        "chunk_ops": chunk_ops,
        "tau_done_at": tau_done_at,
        "tau_blocks": tau_blocks,
    }

    # per-core planes: gather idx plane + host-built seg matrices
    # seg stream layout: DRAM [128, n_ops*128] bf16; op i slice
    # [:, i*128:(i+1)*128] = seg_i[entry_partition, dest_rank] = scale
    per_core = []
    for c in range(N_CORES):
        run, rank, q, sc = cores[c]
        # position within run (entries already sorted by run)
        first = np.searchsorted(run, run, side="left")
        posin = np.arange(len(run)) - first
        pos = run_start[run] + posin
        gidx = np.zeros(S128, np.int16)
        gidx[pos] = q.astype(np.int16)
        opid = op_base[run] + pos // 128 - run_j0[run]
        segs = np.zeros((P, n_ops, P), ml_dtypes.bfloat16)
        segs[pos % 128, opid, rank] = sc.astype(ml_dtypes.bfloat16)
        per_core.append({
            "gidx": _idx_plane16(gidx, S128),
            "segs": np.ascontiguousarray(segs.reshape(P, n_ops * P)),
        })
    return layout, per_core


# --------------------------------------------------------------------------
# Device program
# --------------------------------------------------------------------------

def _build(layout):
    bf16 = mybir.dt.bfloat16
    f32 = mybir.dt.float32
    i16 = mybir.dt.int16
    S128 = layout["S128"]
    n_ops = layout["n_ops"]
    ops = layout["ops"]
    chunk_ops = layout["chunk_ops"]
    tau_done_at = layout["tau_done_at"]
    tau_blocks = layout["tau_blocks"]
    nchunks = S128 // 128

    nseg_blk = (n_ops + SEGBLK - 1) // SEGBLK

    nc = bacc.Bacc(dynamic_dma_scratch_size=SCRATCH)
    xq_d = nc.declare_dram_parameter("xq", [QROWS, QELEM], bf16, isOutput=False)
    xT_d = nc.declare_dram_parameter("xT", [P, SHARD], bf16, isOutput=False)
    gidx_d = nc.declare_dram_parameter("gidx", [P, S128 // 16], i16, isOutput=False)
    segs_d = nc.declare_dram_parameter("segs", [P, n_ops * P], bf16, isOutput=False)
    ab_d = nc.declare_dram_parameter("Ab", [BLOCKS * D, D], bf16, isOutput=False)
    cwT_d = nc.declare_dram_parameter("CwT", [D, D], bf16, isOutput=False)
    cb_d = nc.declare_dram_parameter("Cb", [1, D], bf16, isOutput=False)
    out_d = nc.declare_dram_parameter("out", [SHARD, D], f32, isOutput=True)

    from contextlib import ExitStack

    with tile.TileContext(nc) as tc, ExitStack() as ctx:
        cpool = ctx.enter_context(tc.tile_pool(name="const", bufs=1))
        gpool = ctx.enter_context(tc.tile_pool(name="gath", bufs=4))
        segpool = ctx.enter_context(tc.tile_pool(name="seg", bufs=3))
        hspool = ctx.enter_context(tc.tile_pool(name="hsb", bufs=2))
        opool = ctx.enter_context(tc.tile_pool(name="outb", bufs=3))
        hpsum = ctx.enter_context(tc.tile_pool(name="hps", bufs=1, space="PSUM"))
        opsum = ctx.enter_context(tc.tile_pool(name="ops", bufs=2, space="PSUM"))

        # ---- constants -------------------------------------------------
        nc.gpsimd.load_library(mlp)
        ones = cpool.tile([1, P], bf16, tag="ones")
        nc.vector.memset(ones[:, :], 1.0)
        cb_t = cpool.tile([1, D], bf16, tag="cb")
        nc.sync.dma_start(out=cb_t[:, :], in_=cb_d[:, :])
        cwT_t = cpool.tile([D, D], bf16, tag="cwT")
        nc.sync.dma_start(out=cwT_t[:, :], in_=cwT_d[:, :])
        xT_t = cpool.tile([P, SHARD], bf16, tag="xT")
        nc.sync.dma_start(out=xT_t[:, :], in_=xT_d[:, :])
        a_tiles = []
        for b in range(BLOCKS):
            at = cpool.tile([D, D], bf16, tag=f"A{b}")
            nc.sync.dma_start(out=at[:, :], in_=ab_d[b * D:(b + 1) * D, :])
            a_tiles.append(at)
        gidx_t = cpool.tile([P, S128 // 16], i16, tag="gidx")
        nc.sync.dma_start(out=gidx_t[:, :], in_=gidx_d[:, :])

        # ---- pipeline --------------------------------------------------
        ncalls = (S128 + GCALL - 1) // GCALL
        cols_per_call = GCALL // 128
        gtiles = [None] * ncalls
        stiles = [None] * nseg_blk

        def issue_gather(k):
            start = k * GCALL
            n = min(GCALL, S128 - start)
            gt = gpool.tile([P, cols_per_call, QELEM], bf16, tag="g")
            nc.gpsimd.dma_gather(
                gt[:, : n // 128, :],
                xq_d[:, :],
                gidx_t[:, start // 16: (start + n) // 16],
                n, n, QELEM,
            )
            gtiles[k] = gt

        def issue_segblk(kb):
            st = segpool.tile([P, SEGBLK * P], bf16, tag="segs")
            a = kb * SEGBLK * P
            w = min(SEGBLK * P, n_ops * P - a)
            nc.sync.dma_start(out=st[:, :w], in_=segs_d[:, a: a + w])
            stiles[kb] = st

        h_tiles = {}  # (tau, b) -> psum tile

        for k in range(min(3, ncalls)):
            issue_gather(k)
        for kb in range(min(2, nseg_blk)):
            issue_segblk(kb)

        for k in range(ncalls):
            j0 = k * cols_per_call
            j1 = min(nchunks, j0 + cols_per_call)
            for j in range(j0, j1):
                gt = gtiles[k]
                col = j - j0
                for oi in chunk_ops[j]:
                    _, tau, b, s, o_start, o_stop = ops[oi]
                    kb, ko = divmod(oi, SEGBLK)
                    if ko == 0 and kb + 2 < nseg_blk and stiles[kb + 2] is None:
                        issue_segblk(kb + 2)
                    if o_start:
                        h_tiles[(tau, b)] = hpsum.tile(
                            [P, P], f32, name=f"h{b}", tag=f"h{b}")
                    nc.tensor.matmul(
                        out=h_tiles[(tau, b)][:, :],
                        lhsT=gt[:, col, s * D:(s + 1) * D],
                        rhs=stiles[kb][:, ko * P:(ko + 1) * P],
                        start=o_start, stop=o_stop,
                    )
                # finished dest tiles: second stage
                for tau in tau_done_at[j]:
                    rows = min(P, SHARD - tau * P)
                    po = opsum.tile([P, D], f32, tag="po")
                    blocks = tau_blocks[tau]
                    hs_list = []
                    for b in blocks:
                        hs = hspool.tile([P, P], bf16, tag=f"hs{b}")
                        nc.scalar.copy(out=hs[:, :], in_=h_tiles.pop((tau, b))[:, :])
                        hs_list.append((b, hs))
                    for i, (b, hs) in enumerate(hs_list):
                        nc.tensor.matmul(
                            out=po[:, :],
                            lhsT=hs[:, :],
                            rhs=a_tiles[b][:, :],
                            start=(i == 0), stop=False,
                        )
                    nc.tensor.matmul(
                        out=po[:rows, :],
                        lhsT=xT_t[:, tau * P: tau * P + rows],
                        rhs=cwT_t[:, :],
                        start=(len(hs_list) == 0), stop=False,
                    )
                    nc.tensor.matmul(
                        out=po[:rows, :],
                        lhsT=ones[:1, :rows],
                        rhs=cb_t[:1, :],
                        start=False, stop=True,
                    )
                    ot = opool.tile([P, D], f32, tag="o")
                    nc.vector.tensor_copy(out=ot[:rows, :], in_=po[:rows, :])
                    nc.sync.dma_start(
                        out=out_d[tau * P: tau * P + rows, :],
                        in_=ot[:rows, :])
            if k + 3 < ncalls:
                issue_gather(k + 3)

    nc.compile()
    return nc


# --------------------------------------------------------------------------
# Entry point
# --------------------------------------------------------------------------

_CACHE = {}


def kernel(x, edge_idx_r1, edge_idx_r2, edge_idx_r3, A_r1, A_r2, A_r3, C_w, C_b):
    global LAST_RESULTS
    x = np.asarray(x, np.float32)
    assert x.shape == (N_NODES, D)
    edges = [(edge_idx_r1, 1), (edge_idx_r2, 2), (edge_idx_r3, 3)]
    layout, per_core = _prep(edges)

    key = (layout["S128"], layout["n_ops"],
           tuple(tuple(o) for o in layout["ops"]))
    if key not in _CACHE:
        _CACHE[key] = _build(layout)
    nc = _CACHE[key]

    xb = x.astype(ml_dtypes.bfloat16)
    xq = np.ascontiguousarray(xb.reshape(QROWS, QELEM))
    ab = np.concatenate([
        np.asarray(A_r1, np.float32),
        np.asarray(A_r2, np.float32),
        np.asarray(A_r3, np.float32),
    ], axis=0).astype(ml_dtypes.bfloat16)
    cwT = np.ascontiguousarray(np.asarray(C_w, np.float32).T).astype(
        ml_dtypes.bfloat16)
    cb = np.asarray(C_b, np.float32).reshape(1, D).astype(ml_dtypes.bfloat16)

    in_maps = []
    for c in range(N_CORES):
        in_maps.append({
            "xq": xq,
            "xT": np.ascontiguousarray(xb[c * SHARD:(c + 1) * SHARD].T),
            "gidx": per_core[c]["gidx"],
            "segs": per_core[c]["segs"],
            "Ab": ab,
            "CwT": cwT,
            "Cb": cb,
        })

    res = run_bass_kernel_spmd(nc, in_maps, list(range(N_CORES)), trace=TRACE)
    LAST_RESULTS = res
    return np.concatenate([r["out"] for r in res.results], axis=0)


# revision 19
# speedup vs baseline: 2.6048x; 1.0517x over previous
"""HGNN layer kernel for 8 Trainium2 NeuronCores — gather + on-chip
segmented-sum version (no dma_scatter_add).

out = x @ C_w.T + C_b + sum_t scatter_add(dest_t, (1/counts_t[dest]) * msg_t)
msg_t[g] = concat(x[src_{g,k}] for k in arity) @ A_t

Why this structure: the previous kernel was bottlenecked by SWDGE
descriptor generation on the gpsimd engine (~8.6 ns/descriptor, serialized)
with gathers AND scatter-adds both paying per-entry descriptors.  This
version removes the scatter side entirely:

  - Per-entry decomposition: agg[d] = sum_b (sum_{e: dest=d, blk=b}
    scale_e * x[src_e]) @ A_b.  The inner weighted sum (h_b) is computed
    ON CHIP with PE matmuls against scaled one-hot "segment" matrices
    built by the DVE (tensor_scalar is_equal*mult against an iota tile),
    accumulating in PSUM per (dest-tile, block).
  - agg tile = sum_b h_b @ A_b, computed with accumulating matmuls that
    also fold in the residual (x @ C_w.T via a host-pre-transposed x
    shard) and the bias (rank-1 ones @ C_b).  Output rows are written
    once, contiguously.  No CCE read-modify-write, no occurrence rounds.
  - Gathers remain SWDGE but with a single index window: x is uploaded
    as xq = x.reshape(25000, 512) bf16 (4 nodes per 1024B row), idx =
    src//4 < 25000 fits int16, and the entry picks its node via the
    src%4 slot slice of the gathered row at matmul time.

Entry stream layout (uniform across cores as SPMD requires): entries
sorted by (tau=dest_local//128, block, slot=src%4, src); per-(tau, block,
slot) run lengths are max'ed over cores; pad entries get idx 0 and
rank -1 (the is_equal one-hot zeroes them out).
"""

import sys

for _p in ("/opt/trn_rl_repo",):
    if _p not in sys.path:
        sys.path.insert(0, _p)

import numpy as np
import ml_dtypes

import concourse.bass as bass
import concourse.bacc as bacc
import concourse.mybir as mybir
import concourse.tile as tile
from concourse.bass_utils import run_bass_kernel_spmd
from concourse.library_config import mlp

N_CORES = 8
D = 128
P = 128
N_NODES = 100000
SHARD = N_NODES // N_CORES   # 12500
NTAU = (SHARD + P - 1) // P  # 98
QROWS = N_NODES // 2         # 50000 packed rows of 2 nodes
QELEM = 2 * D                # 256 bf16 elems = 512B per packed row
QMID = QROWS // 2            # gather base row; idx = q - QMID in [-25000, 25000)
SCRATCH = 16384              # dynamic dma scratch bytes/partition (ring = /16)
GCALL = 1024                 # gather idxs per call (= ring capacity)
SEGBLK = 32                  # seg matrices per DMA load
BLOCKS = 6                   # (t0,k0) (t1,k0) (t1,k1) (t2,k0) (t2,k1) (t2,k2)
NSLOT = 2

TRACE = False
LAST_RESULTS = None


# --------------------------------------------------------------------------
# Host-side preparation
# --------------------------------------------------------------------------

def _idx_plane16(vals, length):
    """int16 plane [128, length//16]: entry i -> [i%16, i//16], replicated
    across the 8 gpsimd cores."""
    assert length % 16 == 0
    v = np.zeros(length, np.int16)
    v[: len(vals)] = vals
    pl = v.reshape(length // 16, 16).T  # [16, C]
    return np.ascontiguousarray(np.tile(pl, (8, 1)))


def _prep(edges):
    """Build the uniform entry-stream layout.

    Returns (layout, per_core):
      layout: S128, run table, per-chunk op lists, per-(tau) completion info
      per_core: gidx plane, rank plane, scale plane
    """
    # per-core entry arrays
    core_tau = [[] for _ in range(N_CORES)]
    core_rank = [[] for _ in range(N_CORES)]
    core_b = [[] for _ in range(N_CORES)]
    core_s = [[] for _ in range(N_CORES)]
    core_q = [[] for _ in range(N_CORES)]
    core_sc = [[] for _ in range(N_CORES)]
    for t, (e, arity) in enumerate(edges):
        e = np.asarray(e)
        g = e.shape[1] // arity
        dest = e[1].reshape(g, arity)[:, 0].astype(np.int64)
        srcs = e[0].reshape(g, arity).astype(np.int64)
        counts = np.bincount(dest, minlength=N_NODES)
        inv = np.zeros(N_NODES, np.float32)
        nz = counts > 0
        inv[nz] = np.float32(1.0) / counts[nz].astype(np.float32)
        blk0 = {0: 0, 1: 1, 2: 3}[t]
        core_of = dest // SHARD
        dl = dest - core_of * SHARD
        sc = inv[dest]
        for c in range(N_CORES):
            sel = np.where(core_of == c)[0]
            for k in range(arity):
                s_k = srcs[sel, k]
                core_tau[c].append(dl[sel] // P)
                core_rank[c].append(dl[sel] % P)
                core_b[c].append(np.full(len(sel), blk0 + k, np.int64))
                core_s[c].append(s_k % NSLOT)
                core_q[c].append(s_k // NSLOT)
                core_sc[c].append(sc[sel])

    cores = []
    NRUN = NTAU * BLOCKS * NSLOT
    seg_len = np.zeros(NRUN, np.int64)
    for c in range(N_CORES):
        tau = np.concatenate(core_tau[c])
        rank = np.concatenate(core_rank[c])
        b = np.concatenate(core_b[c])
        s = np.concatenate(core_s[c])
        q = np.concatenate(core_q[c])
        sc = np.concatenate(core_sc[c])
        run = (tau * BLOCKS + b) * NSLOT + s
        order = np.lexsort((q, run))
        run, rank, q, sc = run[order], rank[order], q[order], sc[order]
        np.maximum(seg_len, np.bincount(run, minlength=NRUN), out=seg_len)
        cores.append((run, rank, q, sc))

    run_start = np.concatenate([[0], np.cumsum(seg_len)])
    S = int(run_start[-1])
    S128 = (S + 127) // 128 * 128

    # ops: per run, one op per chunk it overlaps
    #   op fields: (chunk, tau, b, slot, start, stop)
    run_j0 = run_start[:-1] // 128               # first chunk of run
    run_j1 = (run_start[:-1] + np.maximum(seg_len, 1) - 1) // 128  # last chunk
    op_base = np.zeros(NRUN, np.int64)
    ops = []            # list of (chunk, tau, b, slot, start, stop)
    # group boundaries: group = (tau, b); first/last op across its runs
    n_ops_of_group = {}
    for r in range(NRUN):
        if seg_len[r] == 0:
            continue
        op_base[r] = len(ops)
        tau, rem = divmod(r, BLOCKS * NSLOT)
        b, s = divmod(rem, NSLOT)
        for j in range(int(run_j0[r]), int(run_j1[r]) + 1):
            ops.append([j, tau, b, s, False, False])
            n_ops_of_group.setdefault((tau, b), []).append(len(ops) - 1)
    for (tau, b), idxs in n_ops_of_group.items():
        ops[idxs[0]][4] = True
        ops[idxs[-1]][5] = True
    n_ops = len(ops)

    # per-chunk op lists and per-chunk tau completions
    nchunks = S128 // 128
    chunk_ops = [[] for _ in range(nchunks)]
    for i, op in enumerate(ops):
        chunk_ops[op[0]].append(i)
    tau_last_chunk = np.full(NTAU, -1, np.int64)
    for i, (j, tau, b, s, st, sp) in enumerate(ops):
        tau_last_chunk[tau] = max(tau_last_chunk[tau], j)
    assert (tau_last_chunk >= 0).all(), "every dest tile must have entries"
    tau_done_at = [[] for _ in range(nchunks)]
    for tau in range(NTAU):
        if tau_last_chunk[tau] >= 0:
            tau_done_at[int(tau_last_chunk[tau])].append(tau)
    tau_blocks = {}
    for tau in range(NTAU):
        tau_blocks[tau] = sorted(
            b for (t2, b) in n_ops_of_group.keys() if t2 == tau)

    layout = {
        "S128": S128,
        "n_ops": n_ops,
        "ops": ops,
        "chunk_ops": chunk_ops,
        "tau_done_at": tau_done_at,
        "tau_blocks": tau_blocks,
    }

    # per-core planes: gather idx plane + host-built seg matrices
    # seg stream layout: DRAM [128, n_ops*128] bf16; op i slice
    # [:, i*128:(i+1)*128] = seg_i[entry_partition, dest_rank] = scale
    #
    # Gather indices are CENTERED: idx = q - QMID in [-25000, 25000), with
    # the in_ap base at row QMID.  The Q7 kernel truncates trailing negative
    # indices per call, so the last position of every call is swapped (within
    # its run; run-internal order is free) to hold a non-negative index.
    ncalls = (S128 + GCALL - 1) // GCALL
    per_core = []
    for c in range(N_CORES):
        run, rank, q, sc = cores[c]
        # position within run (entries already sorted by run)
        first = np.searchsorted(run, run, side="left")
        posin = np.arange(len(run)) - first
        pos = run_start[run] + posin
        gidx = np.zeros(S128, np.int16)       # pads: idx 0 -> row QMID
        gidx[pos] = (q - QMID).astype(np.int16)
        posrank = np.full(S128, -1, np.int64)
        posrank[pos] = rank
        posscale = np.zeros(S128, np.float32)
        posscale[pos] = sc
        posrun = np.searchsorted(run_start, np.arange(S128), side="right") - 1
        for k in range(ncalls):
            p_last = min(S128, (k + 1) * GCALL) - 1
            if gidx[p_last] >= 0:
                continue
            r = int(posrun[p_last])
            a, b2 = int(run_start[r]), int(run_start[r + 1])
            span = np.arange(a, b2)
            cand = span[(gidx[a:b2] >= 0) & ((span + 1) % GCALL != 0)]
            assert len(cand), "no non-negative idx available in boundary run"
            p2 = int(cand[0])
            for arr in (gidx, posrank, posscale):
                arr[p_last], arr[p2] = arr[p2], arr[p_last]
        valid = np.where(posrank >= 0)[0]
        opid = op_base[posrun[valid]] + valid // 128 - run_j0[posrun[valid]]
        segs = np.zeros((P, n_ops, P), ml_dtypes.bfloat16)
        segs[valid % 128, opid, posrank[valid]] = \
            posscale[valid].astype(ml_dtypes.bfloat16)
        per_core.append({
            "gidx": _idx_plane16(gidx, S128),
            "segs": np.ascontiguousarray(segs.reshape(P, n_ops * P)),
        })
    return layout, per_core


# --------------------------------------------------------------------------
# Device program
# --------------------------------------------------------------------------

def _build(layout):
    bf16 = mybir.dt.bfloat16
    f32 = mybir.dt.float32
    i16 = mybir.dt.int16
    S128 = layout["S128"]
    n_ops = layout["n_ops"]
    ops = layout["ops"]
    chunk_ops = layout["chunk_ops"]
    tau_done_at = layout["tau_done_at"]
    tau_blocks = layout["tau_blocks"]
    nchunks = S128 // 128

    nseg_blk = (n_ops + SEGBLK - 1) // SEGBLK

    nc = bacc.Bacc(dynamic_dma_scratch_size=SCRATCH)
    xq_d = nc.declare_dram_parameter("xq", [QROWS, QELEM], bf16, isOutput=False)
    xT_d = nc.declare_dram_parameter("xT", [P, SHARD], bf16, isOutput=False)
    gidx_d = nc.declare_dram_parameter("gidx", [P, S128 // 16], i16, isOutput=False)
    segs_d = nc.declare_dram_parameter("segs", [P, n_ops * P], bf16, isOutput=False)
    ab_d = nc.declare_dram_parameter("Ab", [BLOCKS * D, D], bf16, isOutput=False)
    cwT_d = nc.declare_dram_parameter("CwT", [D, D], bf16, isOutput=False)
    cb_d = nc.declare_dram_parameter("Cb", [1, D], bf16, isOutput=False)
    out_d = nc.declare_dram_parameter("out", [SHARD, D], f32, isOutput=True)

    from contextlib import ExitStack

    with tile.TileContext(nc) as tc, ExitStack() as ctx:
        cpool = ctx.enter_context(tc.tile_pool(name="const", bufs=1))
        gpool = ctx.enter_context(tc.tile_pool(name="gath", bufs=4))
        segpool = ctx.enter_context(tc.tile_pool(name="seg", bufs=3))
        hspool = ctx.enter_context(tc.tile_pool(name="hsb", bufs=2))
        opool = ctx.enter_context(tc.tile_pool(name="outb", bufs=3))
        hpsum = ctx.enter_context(tc.tile_pool(name="hps", bufs=1, space="PSUM"))
        opsum = ctx.enter_context(tc.tile_pool(name="ops", bufs=2, space="PSUM"))

        # ---- constants -------------------------------------------------
        # gidx loads first (and split) so gathers can start early
        nc.gpsimd.load_library(mlp)
        gcols = S128 // 16
        gsplit = min(gcols, 2048)
        gidx_a = cpool.tile([P, gsplit], i16, tag="gidxA")
        nc.sync.dma_start(out=gidx_a[:, :], in_=gidx_d[:, :gsplit])
        gidx_b = None
        if gsplit < gcols:
            gidx_b = cpool.tile([P, gcols - gsplit], i16, tag="gidxB")
            nc.sync.dma_start(out=gidx_b[:, :], in_=gidx_d[:, gsplit:])
        ones = cpool.tile([1, P], bf16, tag="ones")
        nc.vector.memset(ones[:, :], 1.0)
        cb_t = cpool.tile([1, D], bf16, tag="cb")
        nc.sync.dma_start(out=cb_t[:, :], in_=cb_d[:, :])
        cwT_t = cpool.tile([D, D], bf16, tag="cwT")
        nc.sync.dma_start(out=cwT_t[:, :], in_=cwT_d[:, :])
        xT_t = cpool.tile([P, SHARD], bf16, tag="xT")
        nc.sync.dma_start(out=xT_t[:, :], in_=xT_d[:, :])
        a_tiles = []
        for b in range(BLOCKS):
            at = cpool.tile([D, D], bf16, tag=f"A{b}")
            nc.sync.dma_start(out=at[:, :], in_=ab_d[b * D:(b + 1) * D, :])
            a_tiles.append(at)

        # ---- pipeline --------------------------------------------------
        ncalls = (S128 + GCALL - 1) // GCALL
        cols_per_call = GCALL // 128
        gtiles = [None] * ncalls
        stiles = [None] * nseg_blk

        def issue_gather(k):
            start = k * GCALL
            n = min(GCALL, S128 - start)
            c0 = start // 16
            c1 = (start + n) // 16
            if c1 <= gsplit:
                idx_ap = gidx_a[:, c0:c1]
            else:
                idx_ap = gidx_b[:, c0 - gsplit: c1 - gsplit]
            gt = gpool.tile([P, cols_per_call, QELEM], bf16, tag="g")
            nc.gpsimd.dma_gather(
                gt[:, : n // 128, :],
                xq_d[QMID:QROWS, :],
                idx_ap,
                n, n, QELEM,
            )
            gtiles[k] = gt

        def issue_segblk(kb):
            st = segpool.tile([P, SEGBLK * P], bf16, tag="segs")
            a = kb * SEGBLK * P
            w = min(SEGBLK * P, n_ops * P - a)
            nc.sync.dma_start(out=st[:, :w], in_=segs_d[:, a: a + w])
            stiles[kb] = st

        h_tiles = {}  # (tau, b) -> psum tile

        for k in range(min(3, ncalls)):
            issue_gather(k)
        for kb in range(min(2, nseg_blk)):
            issue_segblk(kb)

        for k in range(ncalls):
            j0 = k * cols_per_call
            j1 = min(nchunks, j0 + cols_per_call)
            for j in range(j0, j1):
                gt = gtiles[k]
                col = j - j0
                for oi in chunk_ops[j]:
                    _, tau, b, s, o_start, o_stop = ops[oi]
                    kb, ko = divmod(oi, SEGBLK)
                    if ko == 0 and kb + 2 < nseg_blk and stiles[kb + 2] is None:
                        issue_segblk(kb + 2)
                    if o_start:
                        h_tiles[(tau, b)] = hpsum.tile(
                            [P, P], f32, name=f"h{b}", tag=f"h{b}")
                    nc.tensor.matmul(
                        out=h_tiles[(tau, b)][:, :],
                        lhsT=gt[:, col, s * D:(s + 1) * D],
                        rhs=stiles[kb][:, ko * P:(ko + 1) * P],
                        start=o_start, stop=o_stop,
                    )
                # finished dest tiles: second stage
                for tau in tau_done_at[j]:
                    rows = min(P, SHARD - tau * P)
                    po = opsum.tile([P, D], f32, tag="po")
                    blocks = tau_blocks[tau]
                    hs_list = []
                    for b in blocks:
                        hs = hspool.tile([P, P], bf16, tag=f"hs{b}")
                        nc.scalar.copy(out=hs[:, :], in_=h_tiles.pop((tau, b))[:, :])
                        hs_list.append((b, hs))
                    for i, (b, hs) in enumerate(hs_list):
                        nc.tensor.matmul(
                            out=po[:, :],
                            lhsT=hs[:, :],
                            rhs=a_tiles[b][:, :],
                            start=(i == 0), stop=False,
                        )
                    nc.tensor.matmul(
                        out=po[:rows, :],
                        lhsT=xT_t[:, tau * P: tau * P + rows],
                        rhs=cwT_t[:, :],
                        start=(len(hs_list) == 0), stop=False,
                    )
                    nc.tensor.matmul(
                        out=po[:rows, :],
                        lhsT=ones[:1, :rows],
                        rhs=cb_t[:1, :],
                        start=False, stop=True,
                    )
                    ot = opool.tile([P, D], f32, tag="o")
                    nc.vector.tensor_copy(out=ot[:rows, :], in_=po[:rows, :])
                    nc.sync.dma_start(
                        out=out_d[tau * P: tau * P + rows, :],
                        in_=ot[:rows, :])
            if k + 3 < ncalls:
                issue_gather(k + 3)

    nc.compile()
    return nc


# --------------------------------------------------------------------------
# Entry point
# --------------------------------------------------------------------------

_CACHE = {}


def kernel(x, edge_idx_r1, edge_idx_r2, edge_idx_r3, A_r1, A_r2, A_r3, C_w, C_b):
    global LAST_RESULTS
    x = np.asarray(x, np.float32)
    assert x.shape == (N_NODES, D)
    edges = [(edge_idx_r1, 1), (edge_idx_r2, 2), (edge_idx_r3, 3)]
    layout, per_core = _prep(edges)

    key = (layout["S128"], layout["n_ops"],
           tuple(tuple(o) for o in layout["ops"]))
    if key not in _CACHE:
        _CACHE[key] = _build(layout)
    nc = _CACHE[key]

    xb = x.astype(ml_dtypes.bfloat16)
    xq = np.ascontiguousarray(xb.reshape(QROWS, QELEM))
    ab = np.concatenate([
        np.asarray(A_r1, np.float32),
        np.asarray(A_r2, np.float32),
        np.asarray(A_r3, np.float32),
    ], axis=0).astype(ml_dtypes.bfloat16)
    cwT = np.ascontiguousarray(np.asarray(C_w, np.float32).T).astype(
        ml_dtypes.bfloat16)
    cb = np.asarray(C_b, np.float32).reshape(1, D).astype(ml_dtypes.bfloat16)

    in_maps = []
    for c in range(N_CORES):
        in_maps.append({
            "xq": xq,
            "xT": np.ascontiguousarray(xb[c * SHARD:(c + 1) * SHARD].T),
            "gidx": per_core[c]["gidx"],
            "segs": per_core[c]["segs"],
            "Ab": ab,
            "CwT": cwT,
            "Cb": cb,
        })

    res = run_bass_kernel_spmd(nc, in_maps, list(range(N_CORES)), trace=TRACE)
    LAST_RESULTS = res
    return np.concatenate([r["out"] for r in res.results], axis=0)


# revision 26
# speedup vs baseline: 2.6839x; 1.0304x over previous
"""HGNN layer kernel for 8 Trainium2 NeuronCores — gather + on-chip
segmented-sum version (no dma_scatter_add).

out = x @ C_w.T + C_b + sum_t scatter_add(dest_t, (1/counts_t[dest]) * msg_t)
msg_t[g] = concat(x[src_{g,k}] for k in arity) @ A_t

Why this structure: the previous kernel was bottlenecked by SWDGE
descriptor generation on the gpsimd engine (~8.6 ns/descriptor, serialized)
with gathers AND scatter-adds both paying per-entry descriptors.  This
version removes the scatter side entirely:

  - Per-entry decomposition: agg[d] = sum_b (sum_{e: dest=d, blk=b}
    scale_e * x[src_e]) @ A_b.  The inner weighted sum (h_b) is computed
    ON CHIP with PE matmuls against scaled one-hot "segment" matrices
    built by the DVE (tensor_scalar is_equal*mult against an iota tile),
    accumulating in PSUM per (dest-tile, block).
  - agg tile = sum_b h_b @ A_b, computed with accumulating matmuls that
    also fold in the residual (x @ C_w.T via a host-pre-transposed x
    shard) and the bias (rank-1 ones @ C_b).  Output rows are written
    once, contiguously.  No CCE read-modify-write, no occurrence rounds.
  - Seg matrices are precomputed on the host and streamed from DRAM in
    SEGBLK-op blocks (building them on the DVE with tensor_scalar
    is_equal*mult measured ~1.25us/op and became the bottleneck).
  - Gathers remain SWDGE (measured hard floor ~8.56 ns/idx of Q7 desc
    generation, independent of row size) with a single index window:
    x is uploaded as xq = x.reshape(50000, 256) bf16 (2 nodes per 512B
    row), the gather base is row QMID=25000 and idx = src//2 - 25000
    spans [-25000, 25000) in int16.  The entry picks its node via the
    src%2 slot slice of the gathered row at matmul time.  The Q7 kernel
    truncates TRAILING negative indices per call, so host prep swaps a
    non-negative index into the last position of every 1024-idx call
    (order within a run is free).

Entry stream layout (uniform across cores as SPMD requires): entries
sorted by (tau=dest_local//128, block, slot=src%2, src); per-(tau, block,
slot) run lengths are max'ed over cores; pad entries get idx 0 and an
all-zero seg row.
"""

import sys

for _p in ("/opt/trn_rl_repo",):
    if _p not in sys.path:
        sys.path.insert(0, _p)

import numpy as np
import ml_dtypes

import concourse.bass as bass
import concourse.bacc as bacc
import concourse.mybir as mybir
import concourse.tile as tile
from concourse.bass_utils import run_bass_kernel_spmd
from concourse.library_config import mlp

N_CORES = 8
D = 128
P = 128
N_NODES = 100000
SHARD = N_NODES // N_CORES   # 12500
NTAU = (SHARD + P - 1) // P  # 98
QROWS = N_NODES // 2         # 50000 packed rows of 2 nodes
QELEM = 2 * D                # 256 bf16 elems = 512B per packed row
QMID = QROWS // 2            # gather base row; idx = q - QMID in [-25000, 25000)
SCRATCH = 16384              # dynamic dma scratch bytes/partition (ring = /16)
GCALL = 1024                 # gather idxs per call (= ring capacity)
SEGBLK = 32                  # seg matrices per DMA load
BLOCKS = 6                   # (t0,k0) (t1,k0) (t1,k1) (t2,k0) (t2,k1) (t2,k2)
NSLOT = 2

TRACE = False
LAST_RESULTS = None


# --------------------------------------------------------------------------
# Host-side preparation
# --------------------------------------------------------------------------

def _idx_plane16(vals, length):
    """int16 plane [128, length//16]: entry i -> [i%16, i//16], replicated
    across the 8 gpsimd cores."""
    assert length % 16 == 0
    v = np.zeros(length, np.int16)
    v[: len(vals)] = vals
    pl = v.reshape(length // 16, 16).T  # [16, C]
    return np.ascontiguousarray(np.tile(pl, (8, 1)))


def _prep(edges):
    """Build the uniform entry-stream layout.

    Returns (layout, per_core):
      layout: S128, run table, per-chunk op lists, per-(tau) completion info
      per_core: gidx plane, rank plane, scale plane
    """
    # per-core entry arrays
    core_tau = [[] for _ in range(N_CORES)]
    core_rank = [[] for _ in range(N_CORES)]
    core_b = [[] for _ in range(N_CORES)]
    core_s = [[] for _ in range(N_CORES)]
    core_q = [[] for _ in range(N_CORES)]
    core_sc = [[] for _ in range(N_CORES)]
    for t, (e, arity) in enumerate(edges):
        e = np.asarray(e)
        g = e.shape[1] // arity
        dest = e[1].reshape(g, arity)[:, 0].astype(np.int64)
        srcs = e[0].reshape(g, arity).astype(np.int64)
        counts = np.bincount(dest, minlength=N_NODES)
        inv = np.zeros(N_NODES, np.float32)
        nz = counts > 0
        inv[nz] = np.float32(1.0) / counts[nz].astype(np.float32)
        blk0 = {0: 0, 1: 1, 2: 3}[t]
        core_of = dest // SHARD
        dl = dest - core_of * SHARD
        sc = inv[dest]
        for c in range(N_CORES):
            sel = np.where(core_of == c)[0]
            for k in range(arity):
                s_k = srcs[sel, k]
                core_tau[c].append(dl[sel] // P)
                core_rank[c].append(dl[sel] % P)
                core_b[c].append(np.full(len(sel), blk0 + k, np.int64))
                core_s[c].append(s_k % NSLOT)
                core_q[c].append(s_k // NSLOT)
                core_sc[c].append(sc[sel])

    cores = []
    NRUN = NTAU * BLOCKS
    seg_len = np.zeros(NRUN, np.int64)
    for c in range(N_CORES):
        tau = np.concatenate(core_tau[c])
        rank = np.concatenate(core_rank[c])
        b = np.concatenate(core_b[c])
        s = np.concatenate(core_s[c])
        q = np.concatenate(core_q[c])
        sc = np.concatenate(core_sc[c])
        run = tau * BLOCKS + b
        order = np.lexsort((q, run))
        run, rank, q, sc, s = (run[order], rank[order], q[order],
                               sc[order], s[order])
        np.maximum(seg_len, np.bincount(run, minlength=NRUN), out=seg_len)
        cores.append((run, rank, q, sc, s))

    run_start = np.concatenate([[0], np.cumsum(seg_len)])
    S = int(run_start[-1])
    S128 = (S + 127) // 128 * 128

    # ops: per run, one op per chunk it overlaps
    #   op fields: (chunk, tau, b, slot, start, stop)
    run_j0 = run_start[:-1] // 128               # first chunk of run
    run_j1 = (run_start[:-1] + np.maximum(seg_len, 1) - 1) // 128  # last chunk
    op_base = np.zeros(NRUN, np.int64)
    ops = []            # list of (chunk, tau, b, slot, start, stop)
    # group boundaries: group = (tau, b); first/last op across its runs
    n_ops_of_group = {}
    for r in range(NRUN):
        if seg_len[r] == 0:
            continue
        op_base[r] = len(ops)
        tau, b = divmod(r, BLOCKS)
        for j in range(int(run_j0[r]), int(run_j1[r]) + 1):
            ops.append([j, tau, b, 0, False, False])
            n_ops_of_group.setdefault((tau, b), []).append(len(ops) - 1)
    for (tau, b), idxs in n_ops_of_group.items():
        ops[idxs[0]][4] = True
        ops[idxs[-1]][5] = True
    n_ops = len(ops)

    # per-chunk op lists and per-chunk tau completions
    nchunks = S128 // 128
    chunk_ops = [[] for _ in range(nchunks)]
    for i, op in enumerate(ops):
        chunk_ops[op[0]].append(i)
    tau_last_chunk = np.full(NTAU, -1, np.int64)
    for i, (j, tau, b, s, st, sp) in enumerate(ops):
        tau_last_chunk[tau] = max(tau_last_chunk[tau], j)
    assert (tau_last_chunk >= 0).all(), "every dest tile must have entries"
    tau_done_at = [[] for _ in range(nchunks)]
    for tau in range(NTAU):
        if tau_last_chunk[tau] >= 0:
            tau_done_at[int(tau_last_chunk[tau])].append(tau)
    tau_blocks = {}
    for tau in range(NTAU):
        tau_blocks[tau] = sorted(
            b for (t2, b) in n_ops_of_group.keys() if t2 == tau)

    layout = {
        "S128": S128,
        "n_ops": n_ops,
        "ops": ops,
        "chunk_ops": chunk_ops,
        "tau_done_at": tau_done_at,
        "tau_blocks": tau_blocks,
    }

    # per-core planes: gather idx plane + host-built seg matrices
    # seg stream layout: DRAM [128, n_ops*128] bf16; op i slice
    # [:, i*128:(i+1)*128] = seg_i[entry_partition, dest_rank] = scale
    #
    # Gather indices are CENTERED: idx = q - QMID in [-25000, 25000), with
    # the in_ap base at row QMID.  The Q7 kernel truncates trailing negative
    # indices per call, so the last position of every call is swapped (within
    # its run; run-internal order is free) to hold a non-negative index.
    ncalls = (S128 + GCALL - 1) // GCALL
    per_core = []
    for c in range(N_CORES):
        run, rank, q, sc, s = cores[c]
        # position within run (entries already sorted by run)
        first = np.searchsorted(run, run, side="left")
        posin = np.arange(len(run)) - first
        pos = run_start[run] + posin
        gidx = np.zeros(S128, np.int16)       # pads: idx 0 -> row QMID
        gidx[pos] = (q - QMID).astype(np.int16)
        posrank = np.full(S128, -1, np.int64)
        posrank[pos] = rank
        posscale = np.zeros(S128, np.float32)
        posscale[pos] = sc
        posslot = np.zeros(S128, np.int64)
        posslot[pos] = s
        posrun = np.searchsorted(run_start, np.arange(S128), side="right") - 1
        for k in range(ncalls):
            p_last = min(S128, (k + 1) * GCALL) - 1
            if gidx[p_last] >= 0:
                continue
            r = int(posrun[p_last])
            a, b2 = int(run_start[r]), int(run_start[r + 1])
            span = np.arange(a, b2)
            cand = span[(gidx[a:b2] >= 0) & ((span + 1) % GCALL != 0)]
            assert len(cand), "no non-negative idx available in boundary run"
            p2 = int(cand[0])
            for arr in (gidx, posrank, posscale, posslot):
                arr[p_last], arr[p2] = arr[p2], arr[p_last]
        valid = np.where(posrank >= 0)[0]
        opid = op_base[posrun[valid]] + valid // 128 - run_j0[posrun[valid]]
        # two seg matrices per op: slot 0 at column 2*op, slot 1 at 2*op+1
        segs = np.zeros((P, 2 * n_ops, P), ml_dtypes.bfloat16)
        segs[valid % 128, 2 * opid + posslot[valid], posrank[valid]] = \
            posscale[valid].astype(ml_dtypes.bfloat16)
        per_core.append({
            "gidx": _idx_plane16(gidx, S128),
            "segs": np.ascontiguousarray(segs.reshape(P, 2 * n_ops * P)),
        })
    return layout, per_core


# --------------------------------------------------------------------------
# Device program
# --------------------------------------------------------------------------

def _build(layout):
    bf16 = mybir.dt.bfloat16
    f32 = mybir.dt.float32
    i16 = mybir.dt.int16
    S128 = layout["S128"]
    n_ops = layout["n_ops"]
    ops = layout["ops"]
    chunk_ops = layout["chunk_ops"]
    tau_done_at = layout["tau_done_at"]
    tau_blocks = layout["tau_blocks"]
    nchunks = S128 // 128

    n_segs = 2 * n_ops
    nseg_blk = (n_segs + SEGBLK - 1) // SEGBLK

    nc = bacc.Bacc(dynamic_dma_scratch_size=SCRATCH)
    xq_d = nc.declare_dram_parameter("xq", [QROWS, QELEM], bf16, isOutput=False)
    xT_d = nc.declare_dram_parameter("xT", [P, SHARD], bf16, isOutput=False)
    gidx_d = nc.declare_dram_parameter("gidx", [P, S128 // 16], i16, isOutput=False)
    segs_d = nc.declare_dram_parameter("segs", [P, n_segs * P], bf16, isOutput=False)
    ab_d = nc.declare_dram_parameter("Ab", [BLOCKS * D, D], bf16, isOutput=False)
    cwT_d = nc.declare_dram_parameter("CwT", [D, D], bf16, isOutput=False)
    cb_d = nc.declare_dram_parameter("Cb", [1, D], bf16, isOutput=False)
    out_d = nc.declare_dram_parameter("out", [SHARD, D], f32, isOutput=True)

    from contextlib import ExitStack

    with tile.TileContext(nc) as tc, ExitStack() as ctx:
        cpool = ctx.enter_context(tc.tile_pool(name="const", bufs=1))
        gpool = ctx.enter_context(tc.tile_pool(name="gath", bufs=4))
        segpool = ctx.enter_context(tc.tile_pool(name="seg", bufs=3))
        hspool = ctx.enter_context(tc.tile_pool(name="hsb", bufs=2))
        opool = ctx.enter_context(tc.tile_pool(name="outb", bufs=3))
        hpsum = ctx.enter_context(tc.tile_pool(name="hps", bufs=1, space="PSUM"))
        opsum = ctx.enter_context(tc.tile_pool(name="ops", bufs=2, space="PSUM"))

        # ---- constants -------------------------------------------------
        # gidx loads first (and split) so gathers can start early
        nc.gpsimd.load_library(mlp)
        gcols = S128 // 16
        gsplit = min(gcols, 2048)
        gidx_a = cpool.tile([P, gsplit], i16, tag="gidxA")
        nc.sync.dma_start(out=gidx_a[:, :], in_=gidx_d[:, :gsplit])
        gidx_b = None
        if gsplit < gcols:
            gidx_b = cpool.tile([P, gcols - gsplit], i16, tag="gidxB")
            nc.sync.dma_start(out=gidx_b[:, :], in_=gidx_d[:, gsplit:])
        ones = cpool.tile([1, P], bf16, tag="ones")
        nc.vector.memset(ones[:, :], 1.0)
        cb_t = cpool.tile([1, D], bf16, tag="cb")
        nc.sync.dma_start(out=cb_t[:, :], in_=cb_d[:, :])
        cwT_t = cpool.tile([D, D], bf16, tag="cwT")
        nc.sync.dma_start(out=cwT_t[:, :], in_=cwT_d[:, :])
        xT_t = cpool.tile([P, SHARD], bf16, tag="xT")
        nc.sync.dma_start(out=xT_t[:, :], in_=xT_d[:, :])
        a_tiles = []
        for b in range(BLOCKS):
            at = cpool.tile([D, D], bf16, tag=f"A{b}")
            nc.sync.dma_start(out=at[:, :], in_=ab_d[b * D:(b + 1) * D, :])
            a_tiles.append(at)

        # ---- pipeline --------------------------------------------------
        ncalls = (S128 + GCALL - 1) // GCALL
        cols_per_call = GCALL // 128
        gtiles = [None] * ncalls
        stiles = [None] * nseg_blk

        def issue_gather(k):
            start = k * GCALL
            n = min(GCALL, S128 - start)
            c0 = start // 16
            c1 = (start + n) // 16
            if c1 <= gsplit:
                idx_ap = gidx_a[:, c0:c1]
            else:
                idx_ap = gidx_b[:, c0 - gsplit: c1 - gsplit]
            gt = gpool.tile([P, cols_per_call, QELEM], bf16, tag="g")
            nc.gpsimd.dma_gather(
                gt[:, : n // 128, :],
                xq_d[QMID:QROWS, :],
                idx_ap,
                n, n, QELEM,
            )
            gtiles[k] = gt

        def issue_segblk(kb):
            st = segpool.tile([P, SEGBLK * P], bf16, tag="segs")
            a = kb * SEGBLK * P
            w = min(SEGBLK * P, n_segs * P - a)
            nc.sync.dma_start(out=st[:, :w], in_=segs_d[:, a: a + w])
            stiles[kb] = st

        h_tiles = {}  # (tau, b) -> psum tile

        for k in range(min(3, ncalls)):
            issue_gather(k)
        for kb in range(min(2, nseg_blk)):
            issue_segblk(kb)

        for k in range(ncalls):
            j0 = k * cols_per_call
            j1 = min(nchunks, j0 + cols_per_call)
            for j in range(j0, j1):
                gt = gtiles[k]
                col = j - j0
                for oi in chunk_ops[j]:
                    _, tau, b, _s, o_start, o_stop = ops[oi]
                    kb, ko = divmod(2 * oi, SEGBLK)
                    if ko == 0 and kb + 2 < nseg_blk and stiles[kb + 2] is None:
                        issue_segblk(kb + 2)
                    if o_start:
                        h_tiles[(tau, b)] = hpsum.tile(
                            [P, P], f32, name=f"h{b}", tag=f"h{b}")
                    nc.tensor.matmul(
                        out=h_tiles[(tau, b)][:, :],
                        lhsT=gt[:, col, 0:D],
                        rhs=stiles[kb][:, ko * P:(ko + 1) * P],
                        start=o_start, stop=False,
                    )
                    nc.tensor.matmul(
                        out=h_tiles[(tau, b)][:, :],
                        lhsT=gt[:, col, D:2 * D],
                        rhs=stiles[kb][:, (ko + 1) * P:(ko + 2) * P],
                        start=False, stop=o_stop,
                    )
                # finished dest tiles: second stage
                for tau in tau_done_at[j]:
                    rows = min(P, SHARD - tau * P)
                    po = opsum.tile([P, D], f32, tag="po")
                    blocks = tau_blocks[tau]
                    hs_list = []
                    for b in blocks:
                        hs = hspool.tile([P, P], bf16, tag=f"hs{b}")
                        nc.scalar.copy(out=hs[:, :], in_=h_tiles.pop((tau, b))[:, :])
                        hs_list.append((b, hs))
                    for i, (b, hs) in enumerate(hs_list):
                        nc.tensor.matmul(
                            out=po[:, :],
                            lhsT=hs[:, :],
                            rhs=a_tiles[b][:, :],
                            start=(i == 0), stop=False,
                        )
                    nc.tensor.matmul(
                        out=po[:rows, :],
                        lhsT=xT_t[:, tau * P: tau * P + rows],
                        rhs=cwT_t[:, :],
                        start=(len(hs_list) == 0), stop=False,
                    )
                    nc.tensor.matmul(
                        out=po[:rows, :],
                        lhsT=ones[:1, :rows],
                        rhs=cb_t[:1, :],
                        start=False, stop=True,
                    )
                    ot = opool.tile([P, D], f32, tag="o")
                    nc.vector.tensor_copy(out=ot[:rows, :], in_=po[:rows, :])
                    nc.sync.dma_start(
                        out=out_d[tau * P: tau * P + rows, :],
                        in_=ot[:rows, :])
            if k + 3 < ncalls:
                issue_gather(k + 3)

    nc.compile()
    return nc


# --------------------------------------------------------------------------
# Entry point
# --------------------------------------------------------------------------

_CACHE = {}


def kernel(x, edge_idx_r1, edge_idx_r2, edge_idx_r3, A_r1, A_r2, A_r3, C_w, C_b):
    global LAST_RESULTS
    x = np.asarray(x, np.float32)
    assert x.shape == (N_NODES, D)
    edges = [(edge_idx_r1, 1), (edge_idx_r2, 2), (edge_idx_r3, 3)]
    layout, per_core = _prep(edges)

    key = (layout["S128"], layout["n_ops"],
           tuple(tuple(o) for o in layout["ops"]))
    if key not in _CACHE:
        _CACHE[key] = _build(layout)
    nc = _CACHE[key]

    xb = x.astype(ml_dtypes.bfloat16)
    xq = np.ascontiguousarray(xb.reshape(QROWS, QELEM))
    ab = np.concatenate([
        np.asarray(A_r1, np.float32),
        np.asarray(A_r2, np.float32),
        np.asarray(A_r3, np.float32),
    ], axis=0).astype(ml_dtypes.bfloat16)
    cwT = np.ascontiguousarray(np.asarray(C_w, np.float32).T).astype(
        ml_dtypes.bfloat16)
    cb = np.asarray(C_b, np.float32).reshape(1, D).astype(ml_dtypes.bfloat16)

    in_maps = []
    for c in range(N_CORES):
        in_maps.append({
            "xq": xq,
            "xT": np.ascontiguousarray(xb[c * SHARD:(c + 1) * SHARD].T),
            "gidx": per_core[c]["gidx"],
            "segs": per_core[c]["segs"],
            "Ab": ab,
            "CwT": cwT,
            "Cb": cb,
        })

    res = run_bass_kernel_spmd(nc, in_maps, list(range(N_CORES)), trace=TRACE)
    LAST_RESULTS = res
    return np.concatenate([r["out"] for r in res.results], axis=0)


# revision 28
# speedup vs baseline: 2.6848x; 1.0003x over previous
"""HGNN layer kernel for 8 Trainium2 NeuronCores — gather + on-chip
segmented-sum version (no dma_scatter_add).

out = x @ C_w.T + C_b + sum_t scatter_add(dest_t, (1/counts_t[dest]) * msg_t)
msg_t[g] = concat(x[src_{g,k}] for k in arity) @ A_t

Why this structure: the previous kernel was bottlenecked by SWDGE
descriptor generation on the gpsimd engine (~8.6 ns/descriptor, serialized)
with gathers AND scatter-adds both paying per-entry descriptors.  This
version removes the scatter side entirely:

  - Per-entry decomposition: agg[d] = sum_b (sum_{e: dest=d, blk=b}
    scale_e * x[src_e]) @ A_b.  The inner weighted sum (h_b) is computed
    ON CHIP with PE matmuls against scaled one-hot "segment" matrices
    built by the DVE (tensor_scalar is_equal*mult against an iota tile),
    accumulating in PSUM per (dest-tile, block).
  - agg tile = sum_b h_b @ A_b, computed with accumulating matmuls that
    also fold in the residual (x @ C_w.T via a host-pre-transposed x
    shard) and the bias (rank-1 ones @ C_b).  Output rows are written
    once, contiguously.  No CCE read-modify-write, no occurrence rounds.
  - Seg matrices are precomputed on the host and streamed from DRAM in
    SEGBLK-op blocks (building them on the DVE with tensor_scalar
    is_equal*mult measured ~1.25us/op and became the bottleneck).
  - Gathers remain SWDGE (measured hard floor ~8.56 ns/idx of Q7 desc
    generation, independent of row size) with a single index window:
    x is uploaded as xq = x.reshape(50000, 256) bf16 (2 nodes per 512B
    row), the gather base is row QMID=25000 and idx = src//2 - 25000
    spans [-25000, 25000) in int16.  The entry picks its node via the
    src%2 slot slice of the gathered row at matmul time.  The Q7 kernel
    truncates TRAILING negative indices per call, so host prep swaps a
    non-negative index into the last position of every 1024-idx call
    (order within a run is free).

Entry stream layout (uniform across cores as SPMD requires): entries
sorted by (tau=dest_local//128, block, src); per-(tau, block) run lengths
are max'ed over cores (slot boundaries vary per core, so every op emits
TWO matmuls — one per slot — each masked by its own host-built seg
matrix; an all-zero seg makes the spare matmul a no-op).  Pad entries
get idx 0 and an all-zero seg row.
"""

import sys

for _p in ("/opt/trn_rl_repo",):
    if _p not in sys.path:
        sys.path.insert(0, _p)

import numpy as np
import ml_dtypes

import concourse.bass as bass
import concourse.bacc as bacc
import concourse.mybir as mybir
import concourse.tile as tile
from concourse.bass_utils import run_bass_kernel_spmd
from concourse.library_config import mlp

N_CORES = 8
D = 128
P = 128
N_NODES = 100000
SHARD = N_NODES // N_CORES   # 12500
NTAU = (SHARD + P - 1) // P  # 98
QROWS = N_NODES // 2         # 50000 packed rows of 2 nodes
QELEM = 2 * D                # 256 bf16 elems = 512B per packed row
QMID = QROWS // 2            # gather base row; idx = q - QMID in [-25000, 25000)
SCRATCH = 16384              # dynamic dma scratch bytes/partition (ring = /16)
GCALL = 1024                 # gather idxs per call (= ring capacity)
SEGBLK = 32                  # seg matrices per DMA load
BLOCKS = 6                   # (t0,k0) (t1,k0) (t1,k1) (t2,k0) (t2,k1) (t2,k2)
NSLOT = 2

TRACE = False
LAST_RESULTS = None


# --------------------------------------------------------------------------
# Host-side preparation
# --------------------------------------------------------------------------

def _idx_plane16(vals, length):
    """int16 plane [128, length//16]: entry i -> [i%16, i//16], replicated
    across the 8 gpsimd cores."""
    assert length % 16 == 0
    v = np.zeros(length, np.int16)
    v[: len(vals)] = vals
    pl = v.reshape(length // 16, 16).T  # [16, C]
    return np.ascontiguousarray(np.tile(pl, (8, 1)))


def _prep(edges):
    """Build the uniform entry-stream layout.

    Returns (layout, per_core):
      layout: S128, run table, per-chunk op lists, per-(tau) completion info
      per_core: gidx plane, rank plane, scale plane
    """
    # per-core entry arrays
    core_tau = [[] for _ in range(N_CORES)]
    core_rank = [[] for _ in range(N_CORES)]
    core_b = [[] for _ in range(N_CORES)]
    core_s = [[] for _ in range(N_CORES)]
    core_q = [[] for _ in range(N_CORES)]
    core_sc = [[] for _ in range(N_CORES)]
    for t, (e, arity) in enumerate(edges):
        e = np.asarray(e)
        g = e.shape[1] // arity
        dest = e[1].reshape(g, arity)[:, 0].astype(np.int64)
        srcs = e[0].reshape(g, arity).astype(np.int64)
        counts = np.bincount(dest, minlength=N_NODES)
        inv = np.zeros(N_NODES, np.float32)
        nz = counts > 0
        inv[nz] = np.float32(1.0) / counts[nz].astype(np.float32)
        blk0 = {0: 0, 1: 1, 2: 3}[t]
        core_of = dest // SHARD
        dl = dest - core_of * SHARD
        sc = inv[dest]
        for c in range(N_CORES):
            sel = np.where(core_of == c)[0]
            for k in range(arity):
                s_k = srcs[sel, k]
                core_tau[c].append(dl[sel] // P)
                core_rank[c].append(dl[sel] % P)
                core_b[c].append(np.full(len(sel), blk0 + k, np.int64))
                core_s[c].append(s_k % NSLOT)
                core_q[c].append(s_k // NSLOT)
                core_sc[c].append(sc[sel])

    cores = []
    NRUN = NTAU * BLOCKS
    seg_len = np.zeros(NRUN, np.int64)
    for c in range(N_CORES):
        tau = np.concatenate(core_tau[c])
        rank = np.concatenate(core_rank[c])
        b = np.concatenate(core_b[c])
        s = np.concatenate(core_s[c])
        q = np.concatenate(core_q[c])
        sc = np.concatenate(core_sc[c])
        run = tau * BLOCKS + b
        order = np.lexsort((q, run))
        run, rank, q, sc, s = (run[order], rank[order], q[order],
                               sc[order], s[order])
        np.maximum(seg_len, np.bincount(run, minlength=NRUN), out=seg_len)
        cores.append((run, rank, q, sc, s))

    run_start = np.concatenate([[0], np.cumsum(seg_len)])
    S = int(run_start[-1])
    S128 = (S + 127) // 128 * 128

    # ops: per run, one op per chunk it overlaps
    #   op fields: (chunk, tau, b, slot, start, stop)
    run_j0 = run_start[:-1] // 128               # first chunk of run
    run_j1 = (run_start[:-1] + np.maximum(seg_len, 1) - 1) // 128  # last chunk
    op_base = np.zeros(NRUN, np.int64)
    ops = []            # list of (chunk, tau, b, slot, start, stop)
    # group boundaries: group = (tau, b); first/last op across its runs
    n_ops_of_group = {}
    for r in range(NRUN):
        if seg_len[r] == 0:
            continue
        op_base[r] = len(ops)
        tau, b = divmod(r, BLOCKS)
        for j in range(int(run_j0[r]), int(run_j1[r]) + 1):
            ops.append([j, tau, b, 0, False, False])
            n_ops_of_group.setdefault((tau, b), []).append(len(ops) - 1)
    for (tau, b), idxs in n_ops_of_group.items():
        ops[idxs[0]][4] = True
        ops[idxs[-1]][5] = True
    n_ops = len(ops)

    # per-chunk op lists and per-chunk tau completions
    nchunks = S128 // 128
    chunk_ops = [[] for _ in range(nchunks)]
    for i, op in enumerate(ops):
        chunk_ops[op[0]].append(i)
    tau_last_chunk = np.full(NTAU, -1, np.int64)
    for i, (j, tau, b, s, st, sp) in enumerate(ops):
        tau_last_chunk[tau] = max(tau_last_chunk[tau], j)
    assert (tau_last_chunk >= 0).all(), "every dest tile must have entries"
    tau_done_at = [[] for _ in range(nchunks)]
    for tau in range(NTAU):
        if tau_last_chunk[tau] >= 0:
            tau_done_at[int(tau_last_chunk[tau])].append(tau)
    tau_blocks = {}
    for tau in range(NTAU):
        tau_blocks[tau] = sorted(
            b for (t2, b) in n_ops_of_group.keys() if t2 == tau)

    layout = {
        "S128": S128,
        "n_ops": n_ops,
        "ops": ops,
        "chunk_ops": chunk_ops,
        "tau_done_at": tau_done_at,
        "tau_blocks": tau_blocks,
    }

    # per-core planes: gather idx plane + host-built seg matrices
    # seg stream layout: DRAM [128, n_ops*128] bf16; op i slice
    # [:, i*128:(i+1)*128] = seg_i[entry_partition, dest_rank] = scale
    #
    # Gather indices are CENTERED: idx = q - QMID in [-25000, 25000), with
    # the in_ap base at row QMID.  The Q7 kernel truncates trailing negative
    # indices per call, so the last position of every call is swapped (within
    # its run; run-internal order is free) to hold a non-negative index.
    ncalls = (S128 + GCALL - 1) // GCALL
    per_core = []
    for c in range(N_CORES):
        run, rank, q, sc, s = cores[c]
        # position within run (entries already sorted by run)
        first = np.searchsorted(run, run, side="left")
        posin = np.arange(len(run)) - first
        pos = run_start[run] + posin
        gidx = np.zeros(S128, np.int16)       # pads: idx 0 -> row QMID
        gidx[pos] = (q - QMID).astype(np.int16)
        posrank = np.full(S128, -1, np.int64)
        posrank[pos] = rank
        posscale = np.zeros(S128, np.float32)
        posscale[pos] = sc
        posslot = np.zeros(S128, np.int64)
        posslot[pos] = s
        posrun = np.searchsorted(run_start, np.arange(S128), side="right") - 1
        for k in range(ncalls):
            p_last = min(S128, (k + 1) * GCALL) - 1
            if gidx[p_last] >= 0:
                continue
            r = int(posrun[p_last])
            a, b2 = int(run_start[r]), int(run_start[r + 1])
            span = np.arange(a, b2)
            cand = span[(gidx[a:b2] >= 0) & ((span + 1) % GCALL != 0)]
            assert len(cand), "no non-negative idx available in boundary run"
            p2 = int(cand[0])
            for arr in (gidx, posrank, posscale, posslot):
                arr[p_last], arr[p2] = arr[p2], arr[p_last]
        valid = np.where(posrank >= 0)[0]
        opid = op_base[posrun[valid]] + valid // 128 - run_j0[posrun[valid]]
        # two seg matrices per op: slot 0 at column 2*op, slot 1 at 2*op+1
        segs = np.zeros((P, 2 * n_ops, P), ml_dtypes.bfloat16)
        segs[valid % 128, 2 * opid + posslot[valid], posrank[valid]] = \
            posscale[valid].astype(ml_dtypes.bfloat16)
        per_core.append({
            "gidx": _idx_plane16(gidx, S128),
            "segs": np.ascontiguousarray(segs.reshape(P, 2 * n_ops * P)),
        })
    return layout, per_core


# --------------------------------------------------------------------------
# Device program
# --------------------------------------------------------------------------

def _build(layout):
    bf16 = mybir.dt.bfloat16
    f32 = mybir.dt.float32
    i16 = mybir.dt.int16
    S128 = layout["S128"]
    n_ops = layout["n_ops"]
    ops = layout["ops"]
    chunk_ops = layout["chunk_ops"]
    tau_done_at = layout["tau_done_at"]
    tau_blocks = layout["tau_blocks"]
    nchunks = S128 // 128

    n_segs = 2 * n_ops
    nseg_blk = (n_segs + SEGBLK - 1) // SEGBLK

    nc = bacc.Bacc(dynamic_dma_scratch_size=SCRATCH)
    xq_d = nc.declare_dram_parameter("xq", [QROWS, QELEM], bf16, isOutput=False)
    xT_d = nc.declare_dram_parameter("xT", [P, SHARD], bf16, isOutput=False)
    gidx_d = nc.declare_dram_parameter("gidx", [P, S128 // 16], i16, isOutput=False)
    segs_d = nc.declare_dram_parameter("segs", [P, n_segs * P], bf16, isOutput=False)
    ab_d = nc.declare_dram_parameter("Ab", [BLOCKS * D, D], bf16, isOutput=False)
    cwT_d = nc.declare_dram_parameter("CwT", [D, D], bf16, isOutput=False)
    cb_d = nc.declare_dram_parameter("Cb", [1, D], bf16, isOutput=False)
    out_d = nc.declare_dram_parameter("out", [SHARD, D], f32, isOutput=True)

    from contextlib import ExitStack

    with tile.TileContext(nc) as tc, ExitStack() as ctx:
        cpool = ctx.enter_context(tc.tile_pool(name="const", bufs=1))
        gpool = ctx.enter_context(tc.tile_pool(name="gath", bufs=4))
        segpool = ctx.enter_context(tc.tile_pool(name="seg", bufs=3))
        hspool = ctx.enter_context(tc.tile_pool(name="hsb", bufs=2))
        opool = ctx.enter_context(tc.tile_pool(name="outb", bufs=3))
        hpsum = ctx.enter_context(tc.tile_pool(name="hps", bufs=1, space="PSUM"))
        opsum = ctx.enter_context(tc.tile_pool(name="ops", bufs=2, space="PSUM"))

        # ---- constants -------------------------------------------------
        # gidx loads first (and split) so gathers can start early
        nc.gpsimd.load_library(mlp)
        gcols = S128 // 16
        gsplit = min(gcols, 512)
        gidx_a = cpool.tile([P, gsplit], i16, tag="gidxA")
        nc.sync.dma_start(out=gidx_a[:, :], in_=gidx_d[:, :gsplit])
        gidx_b = None
        if gsplit < gcols:
            gidx_b = cpool.tile([P, gcols - gsplit], i16, tag="gidxB")
            nc.sync.dma_start(out=gidx_b[:, :], in_=gidx_d[:, gsplit:])
        ones = cpool.tile([1, P], bf16, tag="ones")
        nc.vector.memset(ones[:, :], 1.0)
        cb_t = cpool.tile([1, D], bf16, tag="cb")
        nc.sync.dma_start(out=cb_t[:, :], in_=cb_d[:, :])
        cwT_t = cpool.tile([D, D], bf16, tag="cwT")
        nc.sync.dma_start(out=cwT_t[:, :], in_=cwT_d[:, :])
        xT_t = cpool.tile([P, SHARD], bf16, tag="xT")
        nc.sync.dma_start(out=xT_t[:, :], in_=xT_d[:, :])
        a_tiles = []
        for b in range(BLOCKS):
            at = cpool.tile([D, D], bf16, tag=f"A{b}")
            nc.sync.dma_start(out=at[:, :], in_=ab_d[b * D:(b + 1) * D, :])
            a_tiles.append(at)

        # ---- pipeline --------------------------------------------------
        ncalls = (S128 + GCALL - 1) // GCALL
        cols_per_call = GCALL // 128
        gtiles = [None] * ncalls
        stiles = [None] * nseg_blk

        def issue_gather(k):
            start = k * GCALL
            n = min(GCALL, S128 - start)
            c0 = start // 16
            c1 = (start + n) // 16
            if c1 <= gsplit:
                idx_ap = gidx_a[:, c0:c1]
            else:
                idx_ap = gidx_b[:, c0 - gsplit: c1 - gsplit]
            gt = gpool.tile([P, cols_per_call, QELEM], bf16, tag="g")
            nc.gpsimd.dma_gather(
                gt[:, : n // 128, :],
                xq_d[QMID:QROWS, :],
                idx_ap,
                n, n, QELEM,
            )
            gtiles[k] = gt

        def issue_segblk(kb):
            st = segpool.tile([P, SEGBLK * P], bf16, tag="segs")
            a = kb * SEGBLK * P
            w = min(SEGBLK * P, n_segs * P - a)
            nc.sync.dma_start(out=st[:, :w], in_=segs_d[:, a: a + w])
            stiles[kb] = st

        h_tiles = {}  # (tau, b) -> psum tile

        for k in range(min(3, ncalls)):
            issue_gather(k)
        for kb in range(min(2, nseg_blk)):
            issue_segblk(kb)

        for k in range(ncalls):
            j0 = k * cols_per_call
            j1 = min(nchunks, j0 + cols_per_call)
            for j in range(j0, j1):
                gt = gtiles[k]
                col = j - j0
                for oi in chunk_ops[j]:
                    _, tau, b, _s, o_start, o_stop = ops[oi]
                    kb, ko = divmod(2 * oi, SEGBLK)
                    if ko == 0 and kb + 2 < nseg_blk and stiles[kb + 2] is None:
                        issue_segblk(kb + 2)
                    if o_start:
                        h_tiles[(tau, b)] = hpsum.tile(
                            [P, P], f32, name=f"h{b}", tag=f"h{b}")
                    nc.tensor.matmul(
                        out=h_tiles[(tau, b)][:, :],
                        lhsT=gt[:, col, 0:D],
                        rhs=stiles[kb][:, ko * P:(ko + 1) * P],
                        start=o_start, stop=False,
                    )
                    nc.tensor.matmul(
                        out=h_tiles[(tau, b)][:, :],
                        lhsT=gt[:, col, D:2 * D],
                        rhs=stiles[kb][:, (ko + 1) * P:(ko + 2) * P],
                        start=False, stop=o_stop,
                    )
                # finished dest tiles: second stage
                for tau in tau_done_at[j]:
                    rows = min(P, SHARD - tau * P)
                    po = opsum.tile([P, D], f32, tag="po")
                    blocks = tau_blocks[tau]
                    hs_list = []
                    for b in blocks:
                        hs = hspool.tile([P, P], bf16, tag=f"hs{b}")
                        nc.scalar.copy(out=hs[:, :], in_=h_tiles.pop((tau, b))[:, :])
                        hs_list.append((b, hs))
                    for i, (b, hs) in enumerate(hs_list):
                        nc.tensor.matmul(
                            out=po[:, :],
                            lhsT=hs[:, :],
                            rhs=a_tiles[b][:, :],
                            start=(i == 0), stop=False,
                        )
                    nc.tensor.matmul(
                        out=po[:rows, :],
                        lhsT=xT_t[:, tau * P: tau * P + rows],
                        rhs=cwT_t[:, :],
                        start=(len(hs_list) == 0), stop=False,
                    )
                    nc.tensor.matmul(
                        out=po[:rows, :],
                        lhsT=ones[:1, :rows],
                        rhs=cb_t[:1, :],
                        start=False, stop=True,
                    )
                    ot = opool.tile([P, D], f32, tag="o")
                    nc.vector.tensor_copy(out=ot[:rows, :], in_=po[:rows, :])
                    nc.sync.dma_start(
                        out=out_d[tau * P: tau * P + rows, :],
                        in_=ot[:rows, :])
            if k + 3 < ncalls:
                issue_gather(k + 3)

    nc.compile()
    return nc


# --------------------------------------------------------------------------
# Entry point
# --------------------------------------------------------------------------

_CACHE = {}


def kernel(x, edge_idx_r1, edge_idx_r2, edge_idx_r3, A_r1, A_r2, A_r3, C_w, C_b):
    global LAST_RESULTS
    x = np.asarray(x, np.float32)
    assert x.shape == (N_NODES, D)
    edges = [(edge_idx_r1, 1), (edge_idx_r2, 2), (edge_idx_r3, 3)]
    layout, per_core = _prep(edges)

    key = (layout["S128"], layout["n_ops"],
           tuple(tuple(o) for o in layout["ops"]))
    if key not in _CACHE:
        _CACHE[key] = _build(layout)
    nc = _CACHE[key]

    xb = x.astype(ml_dtypes.bfloat16)
    xq = np.ascontiguousarray(xb.reshape(QROWS, QELEM))
    ab = np.concatenate([
        np.asarray(A_r1, np.float32),
        np.asarray(A_r2, np.float32),
        np.asarray(A_r3, np.float32),
    ], axis=0).astype(ml_dtypes.bfloat16)
    cwT = np.ascontiguousarray(np.asarray(C_w, np.float32).T).astype(
        ml_dtypes.bfloat16)
    cb = np.asarray(C_b, np.float32).reshape(1, D).astype(ml_dtypes.bfloat16)

    in_maps = []
    for c in range(N_CORES):
        in_maps.append({
            "xq": xq,
            "xT": np.ascontiguousarray(xb[c * SHARD:(c + 1) * SHARD].T),
            "gidx": per_core[c]["gidx"],
            "segs": per_core[c]["segs"],
            "Ab": ab,
            "CwT": cwT,
            "Cb": cb,
        })

    res = run_bass_kernel_spmd(nc, in_maps, list(range(N_CORES)), trace=TRACE)
    LAST_RESULTS = res
    return np.concatenate([r["out"] for r in res.results], axis=0)
